# revision 32
# baseline (speedup 1.0000x reference)
"""CartesianDecomposedAttention Trainium2 kernel (v2).

Complex-valued MHA (B=8, S=512, D=1024, H=16, Dh=64) decomposed into real
arithmetic, data-parallel over batch across 8 NeuronCores (one batch element
per core, no collectives).

Key structure (v2, rebuilt from the v1 baseline's trace analysis):
  - QKV/WO projections: 3-matmul Karatsuba complex products; combines are
    DVE subs reading PSUM f32 directly and writing bf16 (rounding AFTER the
    subtraction - more accurate and cheaper than staging via ACT copies).
  - RoPE writes q/k tiles in a dh-STACKED layout: Mq_h=[qr_h;qi_h],
    Sre_h=[kr_h;ki_h], Sim_h=[-ki_h;kr_h] so the scores matmuls contract
    over the full 128 partitions (one pass per Re/Im per head per t-tile,
    half the PE passes of the 64-contraction variant).
  - Softmax: exp comes straight off the ACT Exp table (PSUM in, bf16 out);
    the act-root is restricted to {exp_and_others, trig_and_small} and ACT
    ops are grouped so there are exactly 2 table loads per head pair.
    Phase rotors: ADD_RANGE_WRAP into [-pi,pi], sin via table,
    cos = sin(pi/2 - |x|).
  - Softmax denominator: ones[128,128]-stationary matmul accumulating over
    t-tiles gives D[s] broadcast across all 128 PSUM rows; one DVE
    reciprocal produces a full-width rb tile (no partition broadcasts).
  - AV: stationary packs [vr|vi] / [vn|vr] per head so or/oi accumulate in
    ONE psum per head (2 passes per (head, t-tile) instead of 4); the o_i
    e-rows come out pair-swapped, compensated by a host-side row
    permutation of wo_i.
  - Output written bf16 (host upcasts), halving the tail DMA.

Matmul operands bf16 (fp32 PSUM accumulation); softmax trig fp32.
"""

import os
import sys

sys.path.insert(0, "/opt/trn_rl_repo")

import math

import ml_dtypes
import numpy as np

import concourse.bass as bass
import concourse.mybir as mybir
import concourse.tile as tile
from concourse import bacc
from concourse.dve_ops import ADD_RANGE_WRAP
from bass_rust import add_dep_helper

BF16 = ml_dtypes.bfloat16

B, S, DM, H, DH = 8, 512, 1024, 16, 64
NK = DM // 128          # 8 contraction chunks of 128
NP = H // 2             # 8 head pairs
NST = S // 128          # 4 t-tiles
SCALE = 1.0 / math.sqrt(DH)
TWO_PI = 2.0 * math.pi
SSC = 1.0 - 1e-6

MM_DT = mybir.dt.bfloat16
TAB_DT = mybir.dt.bfloat16
F32 = mybir.dt.float32


def build_body(nc, tc, io):
    AF = mybir.ActivationFunctionType
    ALU = mybir.AluOpType
    V = nc.vector
    G = nc.gpsimd
    A = nc.scalar
    PE = nc.tensor

    out = io["out"]

    const = tc.alloc_tile_pool(name="const", bufs=1)
    ps = tc.alloc_tile_pool(name="ps", bufs=3, space="PSUM")
    psc = tc.alloc_tile_pool(name="psc", bufs=3, space="PSUM")
    pav = tc.alloc_tile_pool(name="pav", bufs=2, space="PSUM")
    qk_pool = tc.alloc_tile_pool(name="qk", bufs=1)
    v_pool = tc.alloc_tile_pool(name="vp", bufs=1)
    o_pool = tc.alloc_tile_pool(name="op", bufs=1)
    work = tc.alloc_tile_pool(name="wk", bufs=1)
    pool_x = tc.alloc_tile_pool(name="pool_x", bufs=1)

    # ---------------- prologue DMAs (priority order) ----------------
    # pair-0 q-etile weights first (unblocks the very first matmul), x
    # pieces split in half per stream so chunk-0 matmuls start early.
    wq_cur = {}

    def load_wq(which, j, eng=None):
        et = j if which == "q" else NK + j
        for nm in ("r", "i", "s"):
            t = pool_x.tile([128, NK * 128], MM_DT, name=f"w_{nm}_{which}{j}",
                          tag=f"wst_{nm}", bufs=2)
            (eng or nc.sync).dma_start(t[:], io[f"wq_{nm}"][et])
            wq_cur[(which, nm)] = t

    load_wq("q", 0)
    x_sb = {"r": [None, None], "i": [None, None], "s": [None, None]}
    for nm, piece, eng in (("r", 0, nc.scalar), ("i", 0, nc.gpsimd),
                           ("r", 1, nc.scalar), ("i", 1, nc.gpsimd)):
        t = pool_x.tile([128, 4 * S], MM_DT, name=f"x_{nm}_{piece}",
                        tag=f"x{nm}{piece}", bufs=1)
        eng.dma_start(t[:], io[f"x_{nm}_{piece}"][:])
        x_sb[nm][piece] = t
    load_wq("k", 0, eng=nc.scalar)

    # ---------------- constants ----------------
    cos_sb = const.tile([128, S], TAB_DT, name="cos_sb")
    sin_sb = const.tile([128, S], TAB_DT, name="sin_sb")
    cosq_sb = const.tile([128, S], TAB_DT, name="cosq_sb")
    sinq_sb = const.tile([128, S], TAB_DT, name="sinq_sb")
    nc.gpsimd.dma_start(cosq_sb[:], io["cos_q"][:])
    nc.gpsimd.dma_start(sinq_sb[:], io["sin_q"][:])
    nc.gpsimd.dma_start(cos_sb[:], io["cos"][:])
    nc.gpsimd.dma_start(sin_sb[:], io["sin"][:])
    for piece in range(2):
        t = pool_x.tile([128, 4 * S], MM_DT, name=f"x_s_{piece}",
                        tag=f"xs{piece}", bufs=1)
        nc.gpsimd.dma_start(t[:], io[f"x_s_{piece}"][:])
        x_sb["s"][piece] = t
    ones_sb = const.tile([128, 128], MM_DT, name="ones_sb")
    G.memset(ones_sb[:], 1.0)
    b_hpi = const.tile([128, 1], F32, name="b_hpi")
    G.memset(b_hpi[:], math.pi / 2)

    def xch(nm, k):
        return x_sb[nm][k // 4][:, (k % 4) * S:(k % 4 + 1) * S]

    # ---------------- QK projection + RoPE (stacked layout) -------------
    def qk_pair(j):
        """Emit projection + rope for pair j; returns (Mq, Sre, Sim)."""
        res = {}
        for which in ("q", "k"):
            if not (which == "q" and j == 0):
                if not (which == "k" and j == 0):
                    load_wq(which, j)
            w_r = wq_cur[(which, "r")]
            w_i = wq_cur[(which, "i")]
            w_n = wq_cur[(which, "s")]  # holds -wi (4-mult complex)
            p_ar = ps.tile([128, 512], F32, name=f"qk1_{which}{j}", tag="ps")
            p_ai = ps.tile([128, 512], F32, name=f"qk2_{which}{j}", tag="ps")
            for k in range(NK):
                ksl = slice(k * 128, (k + 1) * 128)
                PE.matmul(p_ar[:], w_r[:, ksl], xch("r", k), start=(k == 0),
                          stop=False)
            for k in range(NK):
                ksl = slice(k * 128, (k + 1) * 128)
                PE.matmul(p_ar[:], w_n[:, ksl], xch("i", k), start=False,
                          stop=(k == NK - 1))
            for k in range(NK):
                ksl = slice(k * 128, (k + 1) * 128)
                PE.matmul(p_ai[:], w_i[:, ksl], xch("r", k), start=(k == 0),
                          stop=False)
            for k in range(NK):
                ksl = slice(k * 128, (k + 1) * 128)
                PE.matmul(p_ai[:], w_r[:, ksl], xch("i", k), start=False,
                          stop=(k == NK - 1))
            c_t = cosq_sb if which == "q" else cos_sb
            s_t = sinq_sb if which == "q" else sin_sb
            t1 = work.tile([128, 512], MM_DT, name=f"t1_{which}{j}", tag="t1",
                           bufs=1)
            t2 = work.tile([128, 512], MM_DT, name=f"t2_{which}{j}", tag="t2",
                           bufs=1)
            t3 = work.tile([128, 512], MM_DT, name=f"t3_{which}{j}", tag="t3",
                           bufs=1)
            t4 = work.tile([128, 512], MM_DT, name=f"t4_{which}{j}", tag="t4",
                           bufs=1)
            V.tensor_mul(t1[:], p_ar[:], c_t[:])
            V.tensor_mul(t2[:], p_ai[:], s_t[:])
            V.tensor_mul(t3[:], p_ar[:], s_t[:])
            V.tensor_mul(t4[:], p_ai[:], c_t[:])
            tiles = []
            for half in range(2):
                hs = slice(half * 64, (half + 1) * 64)
                if which == "q":
                    mq = qk_pool.tile([128, 512], MM_DT,
                                      name=f"mq_{j}_{half}", tag="mq", bufs=5)
                    V.tensor_sub(mq[0:64, :], t1[hs, :], t2[hs, :])
                    V.tensor_add(mq[64:128, :], t3[hs, :], t4[hs, :])
                    tiles.append(mq)
                else:
                    sre = qk_pool.tile([128, 512], MM_DT,
                                       name=f"sre_{j}_{half}", tag="sre",
                                       bufs=5)
                    sim = qk_pool.tile([128, 512], MM_DT,
                                       name=f"sim_{j}_{half}", tag="sim",
                                       bufs=5)
                    V.tensor_sub(sre[0:64, :], t1[hs, :], t2[hs, :])
                    V.tensor_add(sre[64:128, :], t3[hs, :], t4[hs, :])
                    V.tensor_scalar_mul(sim[0:64, :], sre[64:128, :], -1.0)
                    V.tensor_scalar_mul(sim[64:128, :], sre[0:64, :], 1.0)
                    tiles.append((sre, sim))
            res[which] = tiles
        return res["q"], [t[0] for t in res["k"]], [t[1] for t in res["k"]]

    # ---------------- V projection into packed AV layouts ----------------
    # v_AB[tt] blocks per head h: even h -> [vr|vi], odd h -> [vi|vr]
    # v_CD[tt] blocks per head h: even h -> [vn|vr], odd h -> [vr|vn]
    v_AB = [v_pool.tile([128, 2048], MM_DT, name=f"vab_{tt}", tag="vab",
                        bufs=NST) for tt in range(NST)]
    v_CD = [v_pool.tile([128, 2048], MM_DT, name=f"vcd_{tt}", tag="vcd",
                        bufs=NST) for tt in range(NST)]

    wv_cur = {}

    def load_wv(n):
        for nm, eng in (("r", nc.gpsimd), ("i", nc.gpsimd), ("s", nc.scalar)):
            t = pool_x.tile([128, NK * 512], MM_DT, name=f"wv_{nm}_{n}",
                          tag=f"wv{nm}", bufs=1)
            eng.dma_start(t[:], io[f"wv_{nm}"][n])
            wv_cur[nm] = t

    def quad4(ap1024, q):
        # [128,1024] -> [128,4,64] selecting 64-col blocks at base q*64,
        # stride 256
        return ap1024.rearrange("p (k q i) -> p k q i", k=4, q=4, i=64)[
            :, :, q, :]

    def half8(ap512, q):
        # psum [128,512] -> [128,4,64] selecting head-blocks at base q*64,
        # stride 128 (q=0: even heads, q=1: odd heads)
        return ap512.rearrange("p (k q i) -> p k q i", k=4, q=2, i=64)[
            :, :, q, :]

    def v_chunk(n, st):
        def wvch(nm, k):
            return wv_cur[nm][:, k * 512:(k + 1) * 512]

        ssl = slice(st * 128, (st + 1) * 128)
        p1 = ps.tile([128, 512], F32, name=f"vp1_{st}_{n}", tag="ps")
        p2 = ps.tile([128, 512], F32, name=f"vp2_{st}_{n}", tag="ps")
        p3 = ps.tile([128, 512], F32, name=f"vp3_{st}_{n}", tag="ps")
        for k in range(NK):
            PE.matmul(p1[:], xch("r", k)[:, ssl], wvch("r", k),
                      start=(k == 0), stop=(k == NK - 1))
        for k in range(NK):
            PE.matmul(p2[:], xch("i", k)[:, ssl], wvch("i", k),
                      start=(k == 0), stop=(k == NK - 1))
        for k in range(NK):
            PE.matmul(p3[:], xch("s", k)[:, ssl], wvch("s", k),
                      start=(k == 0), stop=(k == NK - 1))
        ab = v_AB[st][:, n * 1024:(n + 1) * 1024]
        cd = v_CD[st][:, n * 1024:(n + 1) * 1024]
        pc = work.tile([128, 512], F32, name=f"vpc_{st}_{n}", tag="stg",
                       bufs=2)
        V.tensor_scalar_mul(pc[:], p1[:], 1.0)
        # vr = p1 - p2 into AB (even:+0, odd:+64)
        V.tensor_sub(quad4(ab, 0), half8(pc[:], 0), half8(p2[:], 0))
        V.tensor_sub(quad4(ab, 3), half8(pc[:], 1), half8(p2[:], 1))
        # vi = p3 - p1 - p2 into AB (even:+64, odd:+0)
        tf = work.tile([128, 512], F32, name=f"vtf_{st}_{n}", tag="tf",
                       bufs=2)
        V.tensor_sub(tf[:], p3[:], pc[:])
        V.tensor_sub(quad4(ab, 1), half8(tf[:], 0), half8(p2[:], 0))
        V.tensor_sub(quad4(ab, 2), half8(tf[:], 1), half8(p2[:], 1))
        # vr copies into CD (even:+64, odd:+0)
        V.tensor_scalar_mul(quad4(cd, 1), quad4(ab, 0), 1.0)
        V.tensor_scalar_mul(quad4(cd, 2), quad4(ab, 3), 1.0)
        # vn = -vi into CD (even:+0, odd:+64)
        V.tensor_scalar_mul(quad4(cd, 0), quad4(ab, 1), -1.0)
        V.tensor_scalar_mul(quad4(cd, 3), quad4(ab, 2), -1.0)

    # ---------------- output accumulators ----------------
    o_r = [o_pool.tile([128, S], MM_DT, name=f"o_r_{j}", tag="o_r", bufs=NP)
           for j in range(NP)]
    o_i = [o_pool.tile([128, S], MM_DT, name=f"o_i_{j}", tag="o_i", bufs=NP)
           for j in range(NP)]
    o_s = [o_pool.tile([128, S], MM_DT, name=f"o_s_{j}", tag="o_s", bufs=NP)
           for j in range(NP)]

    # ---------------- scores + softmax + AV ----------------
    av_state = {}

    def scores_and_av(j, qk, prev):
        """Emit pair j's scores/softmax; interleave pair prev's denominator
        and AV matmuls into the same PE window."""
        Mq, Sre, Sim = qk
        ebs = [work.tile([128, NST * 512], MM_DT, name=f"eb_{j}_{h}",
                         tag="eb", bufs=2) for h in range(2)]
        phs = [work.tile([128, 2048], F32, name=f"ph_{j}_{h}", tag="ph",
                         bufs=2) for h in range(2)]

        rb_prev = None
        if prev is not None:
            rb_prev = av_state[prev][3]
            ps_o = [pav.tile([128, 512], F32, name=f"av_{prev}_{h}",
                             tag="pav") for h in range(2)]
            Wr_p, Wi_p = av_state[prev][1], av_state[prev][2]

        # scores matmuls (+ AV(prev) interleaved per tt)
        for tt in range(NST):
            tsl = slice(tt * 128, (tt + 1) * 128)
            csl = slice(tt * 512, (tt + 1) * 512)
            for half in range(2):
                ps_re = psc.tile([128, 512], F32, name=f"re_{j}_{half}_{tt}",
                                 tag="psc")
                ps_im = psc.tile([128, 512], F32, name=f"im_{j}_{half}_{tt}",
                                 tag="psc")
                PE.matmul(ps_re[:], Sre[half][:, tsl], Mq[half][:],
                          start=True, stop=True)
                PE.matmul(ps_im[:], Sim[half][:, tsl], Mq[half][:],
                          start=True, stop=True)
                A.activation(ebs[half][:, csl], ps_re[:], AF.Exp,
                             scale=1.0)
                V._custom_dve(ADD_RANGE_WRAP,
                              out=phs[half][:, tt * 512:(tt + 1) * 512],
                              in0=ps_im[:], s0=0.0, s1=math.pi, imm2=TWO_PI)
            if prev is not None:
                h2 = 2 * prev
                for half in range(2):
                    blk = slice((h2 + half) * 128, (h2 + half + 1) * 128)
                    PE.matmul(ps_o[half][:], v_AB[tt][:, blk],
                              Wr_p[half][:, csl], start=(tt == 0), stop=False,
                              skip_group_check=True)
                    PE.matmul(ps_o[half][:], v_CD[tt][:, blk],
                              Wi_p[half][:, csl], start=False,
                              stop=(tt == NST - 1), skip_group_check=True)

        # denominator for THIS pair right after its exps (fills the PE
        # window before trig completes; recip gets a full iteration of lead)
        rb_self = []
        for half in range(2):
            ps_d = psc.tile([128, 512], F32, name=f"dn_{j}_{half}",
                            tag="psc")
            for tt in range(NST):
                PE.matmul(ps_d[:], ones_sb[:],
                          ebs[half][:, tt * 512:(tt + 1) * 512],
                          start=(tt == 0), stop=(tt == NST - 1))
            rb = work.tile([128, 512], F32, name=f"rb_{j}_{half}",
                           tag="rb", bufs=3)
            V.reciprocal_approx_fast(out=rb[:], in_=ps_d[:])
            rb_self.append(rb)
        # trig (single table switch per pair: all exps above, sins below).
        # A value-preserving mark on each ph tile makes every sin's input
        # depend on the pair's last exps, so the scheduler can't interleave
        # exps and sins (would thrash ACT table loads).
        if j < NP - 1:
            tok = work.tile([128, 1], F32, name=f"tok_{j}", tag="tok",
                            bufs=2)
            V.scalar_tensor_tensor(tok[:], ebs[0][:, 2047:2048], 0.0,
                                   ebs[1][:, 2047:2048], ALU.mult, ALU.add)
            for half in range(2):
                for c in range(2):
                    cc = c * 1024
                    V.scalar_tensor_tensor(phs[half][:, cc:cc + 1], tok[:],
                                           0.0, phs[half][:, cc:cc + 1],
                                           ALU.mult, ALU.add)
        s1 = [[None] * 2 for _ in range(2)]
        c1 = [[None] * 2 for _ in range(2)]
        for half in range(2):
            for c in range(2):
                s1t = work.tile([128, 1024], MM_DT, name=f"s1_{j}_{half}_{c}",
                                tag="s1", bufs=2)
                c1t = work.tile([128, 1024], MM_DT, name=f"c1_{j}_{half}_{c}",
                                tag="c1", bufs=2)
                ph = phs[half][:, c * 1024:(c + 1) * 1024]
                A.activation(s1t[:], ph, AF.Sin, scale=SSC)
                A.activation(ph, ph, AF.Abs, scale=1.0)
                A.activation(c1t[:], ph, AF.Sin, bias=b_hpi[:], scale=-1.0)
                s1[half][c] = s1t
                c1[half][c] = c1t
        # W~ = e * (cos, sin); Wr on gpsimd to offload the DVE
        Wr = [work.tile([128, NST * 512], MM_DT, name=f"Wr_{j}_{h}", tag="Wr",
                        bufs=2) for h in range(2)]
        Wi = [work.tile([128, NST * 512], MM_DT, name=f"Wi_{j}_{h}", tag="Wi",
                        bufs=2) for h in range(2)]
        for half in range(2):
            for c in range(2):
                cl = slice(c * 1024, (c + 1) * 1024)
                V.tensor_mul(Wr[half][:, cl], ebs[half][:, cl],
                             c1[half][c][:])
                V.tensor_mul(Wi[half][:, cl], ebs[half][:, cl], s1[half][c][:])

        # AV(prev) eviction
        if prev is not None:
            pj = prev
            V.tensor_mul(o_r[pj][0:64, :], ps_o[0][0:64, :], rb_prev[0][0:64, :])
            V.tensor_mul(o_r[pj][64:128, :], ps_o[1][64:128, :],
                         rb_prev[1][64:128, :])
            V.tensor_mul(o_i[pj][0:64, :], ps_o[1][0:64, :], rb_prev[1][0:64, :])
            V.tensor_mul(o_i[pj][64:128, :], ps_o[0][64:128, :],
                         rb_prev[0][64:128, :])
            osa = work.tile([128, 512], MM_DT, name=f"osa_{pj}", tag="osa",
                            bufs=1)
            V.tensor_scalar_mul(osa[0:64, :], o_i[pj][64:128, :], 1.0)
            V.tensor_scalar_mul(osa[64:128, :], o_i[pj][0:64, :], 1.0)
            V.tensor_add(o_s[pj][:, :], o_r[pj][:, :], osa[:, :])

        av_state[j] = (ebs, Wr, Wi, rb_self)
        if prev is not None:
            del av_state[prev]

    # ---------------- phase 3: output projection (Karatsuba) -------------
    wo_sb = {}

    def load_wo_n0():
        # n=0 halves reuse the wv tag slots (all wv reads done by v(1,3))
        for nm, eng, tag in (("r", nc.sync, "wvr"), ("i", nc.scalar, "wvi"),
                             ("s", nc.gpsimd, "wvs")):
            t = pool_x.tile([128, NK * 512], MM_DT, name=f"wo_{nm}_0",
                            tag=tag, bufs=1)
            eng.dma_start(t[:], io[f"wo_{nm}"][0])
            wo_sb[(nm, 0)] = t

    def load_wo_n1():
        # n=1 halves reuse the x tag slots (x reads done by qk_pair(7))
        slots = {"r": ("xs0", "xs1"), "i": ("xi0", "xi1"),
                 "s": ("xr0", "xr1")}
        for nm, eng in (("r", nc.sync), ("i", nc.gpsimd), ("s", nc.scalar)):
            halves = []
            for piece in range(2):
                t = pool_x.tile([128, 4 * 512], MM_DT,
                                name=f"wo_{nm}_1_{piece}",
                                tag=slots[nm][piece], bufs=1)
                eng.dma_start(t[:], io[f"wo_{nm}"][1][:, piece * 2048:
                                                     (piece + 1) * 2048])
                halves.append(t)
            wo_sb[(nm, 1)] = halves

    def woch(nm, k, n):
        if n == 0:
            return wo_sb[(nm, 0)][:, k * 512:(k + 1) * 512]
        t = wo_sb[(nm, 1)][k // 4]
        kk = k % 4
        return t[:, kk * 512:(kk + 1) * 512]

    def phase3_unit(st, n, pool, upto=NK):
        ssl = slice(st * 128, (st + 1) * 128)
        tag = "ps" if pool is ps else "psc"
        p1 = pool.tile([128, 512], F32, name=f"pj1_{st}_{n}", tag=tag)
        p2 = pool.tile([128, 512], F32, name=f"pj2_{st}_{n}", tag=tag)
        p3 = pool.tile([128, 512], F32, name=f"pj3_{st}_{n}", tag=tag)
        state = {"k": 0}

        def advance(upto2):
            for k in range(state["k"], upto2):
                PE.matmul(p1[:], o_r[k][:, ssl], woch("r", k, n),
                          start=(k == 0), stop=(k == NK - 1))
                PE.matmul(p2[:], o_i[k][:, ssl], woch("i", k, n),
                          start=(k == 0), stop=(k == NK - 1))
                PE.matmul(p3[:], o_s[k][:, ssl], woch("s", k, n),
                          start=(k == 0), stop=(k == NK - 1))
            state["k"] = upto2

        def finish():
            advance(NK)
            to_r = work.tile([128, 512], MM_DT, name=f"otr_{st}_{n}",
                             tag="out_r", bufs=1)
            to_i = work.tile([128, 512], MM_DT, name=f"oti_{st}_{n}",
                             tag="out_i", bufs=1)
            tf3 = work.tile([128, 512], F32, name=f"otf_{st}_{n}",
                            tag="out_f", bufs=1)
            pc3 = work.tile([128, 512], F32, name=f"opc_{st}_{n}",
                            tag="stg", bufs=2)
            V.tensor_scalar_mul(pc3[:], p1[:], 1.0)
            V.tensor_sub(to_r[:], pc3[:], p2[:])
            V.tensor_sub(tf3[:], p3[:], pc3[:])
            V.tensor_sub(to_i[:], tf3[:], p2[:])
            nsl = slice(n * 512, (n + 1) * 512)
            eng_o = [nc.sync, nc.scalar, nc.gpsimd][(st + n) % 3]
            eng_o.dma_start(out[0, ssl, nsl], to_r[:])
            eng_o.dma_start(out[1, ssl, nsl], to_i[:])

        advance(upto)
        return advance, finish

    # ---------------- emission schedule ----------------
    load_wv(0)
    qk0 = qk_pair(0)
    qk1 = qk_pair(1)
    scores_and_av(0, qk0, None)
    for st in range(NST):
        v_chunk(0, st)
    qk_tiles = {1: qk1}
    pre_units = []
    for j in range(1, NP):
        if j + 1 < NP:
            qk_tiles[j + 1] = qk_pair(j + 1)
            if j + 1 == NP - 1:
                load_wo_n1()
        if j == 5:
            load_wo_n0()
        if j == NP - 1:
            # fill the last pair's softmax latency with phase-3 partials
            # (ps pool only - psc is still needed by scores/denom)
            pre_units.append(phase3_unit(0, 0, ps, upto=NK - 2))
        scores_and_av(j, qk_tiles.pop(j), j - 1)
        if j == 1:
            load_wv(1)
        if 1 <= j <= NST:
            v_chunk(1, j - 1)

    # last pair's denominator+AV, interleaved with the first phase-3 units
    for adv, _f in pre_units:
        adv(NK - 1)
    # emit AV for pair 7 (denominator already computed in its scores pass)
    prev = NP - 1
    rb_prev = av_state[prev][3]
    pre_units.append(phase3_unit(1, 0, psc, upto=NK - 1))
    ps_o = [pav.tile([128, 512], F32, name=f"av_{prev}_{h}", tag="pav")
            for h in range(2)]
    Wr_p, Wi_p = av_state[prev][1], av_state[prev][2]
    h2 = 2 * prev
    for tt in range(NST):
        csl = slice(tt * 512, (tt + 1) * 512)
        for half in range(2):
            blk = slice((h2 + half) * 128, (h2 + half + 1) * 128)
            PE.matmul(ps_o[half][:], v_AB[tt][:, blk], Wr_p[half][:, csl],
                      start=(tt == 0), stop=False, skip_group_check=True)
            PE.matmul(ps_o[half][:], v_CD[tt][:, blk], Wi_p[half][:, csl],
                      start=False, stop=(tt == NST - 1),
                      skip_group_check=True)
    pj = prev
    V.tensor_mul(o_r[pj][0:64, :], ps_o[0][0:64, :], rb_prev[0][0:64, :])
    V.tensor_mul(o_r[pj][64:128, :], ps_o[1][64:128, :], rb_prev[1][64:128, :])
    V.tensor_mul(o_i[pj][0:64, :], ps_o[1][0:64, :], rb_prev[1][0:64, :])
    V.tensor_mul(o_i[pj][64:128, :], ps_o[0][64:128, :], rb_prev[0][64:128, :])
    osa = work.tile([128, 512], MM_DT, name=f"osa_{pj}", tag="osa", bufs=1)
    V.tensor_scalar_mul(osa[0:64, :], o_i[pj][64:128, :], 1.0)
    V.tensor_scalar_mul(osa[64:128, :], o_i[pj][0:64, :], 1.0)
    V.tensor_add(o_s[pj][:, :], o_r[pj][:, :], osa[:, :])
    del av_state[prev]

    for _adv, fin in pre_units:
        fin()
    pools3 = [ps, psc]
    i3 = 0
    for n in range(2):
        for st in range(NST):
            if st <= 1 and n == 0:
                continue
            _a, fin = phase3_unit(st, n, pools3[i3 % 2])
            fin()
            i3 += 1

    for p in (pool_x, work, o_pool, v_pool, qk_pool, pav, psc, ps, const):
        p.release()


def _install_act_root():
    """Restrict walrus to the {exp_and_others, trig_and_small} ACT table
    sets so exp and sin are each one load away and nothing else thrashes.
    On any failure, degrade to the default tables (correct, slower)."""
    if os.environ.get("K_NO_ACTFIX"):
        return
    if os.environ.get("BASS_ACT_ROOT_JSON_PATH"):
        return
    try:
        _install_act_root_impl()
    except Exception:
        os.environ["K_NO_ACTFIX"] = "1"


_KEEP_SETS = ("exp_and_others", "trig_and_small")


def _install_act_root_impl():
    import json
    import tempfile
    from neuronxcc.driver.Job import Job
    from neuronxcc.driver.jobs.support.FindActInfo import findActInfoFile

    p = findActInfoFile(Job.getPackageDir(), "gen3")
    src_dir = os.path.dirname(p)
    with open(p) as f:
        d = json.load(f)
    d["act_func_sets"] = [e for e in d["act_func_sets"]
                          if e["name"] in _KEEP_SETS]
    out_dir = tempfile.mkdtemp(prefix="act_expsin_")
    for fn in os.listdir(src_dir):
        sp = os.path.join(src_dir, fn)
        if os.path.isfile(sp) and fn != "act_info.json":
            os.symlink(sp, os.path.join(out_dir, fn))
    with open(os.path.join(out_dir, "act_info.json"), "w") as f:
        json.dump(d, f)
    os.environ["BASS_ACT_ROOT_JSON_PATH"] = os.path.join(out_dir,
                                                         "act_info.json")
    import concourse.hw_specs as hw_specs
    import concourse.bacc as bacc_mod

    orig = hw_specs.get_activation_tables.__wrapped__

    @__import__("functools").cache
    def only_kept(arch):
        full = orig(arch)
        return {k: full[k] for k in _KEEP_SETS}

    hw_specs.get_activation_tables = only_kept
    bacc_mod.get_activation_tables = only_kept


def build_nc():
    _install_act_root()
    nc = bacc.Bacc("TRN2", target_bir_lowering=False, debug=False,
                   enable_asserts=False, num_devices=8)
    io = {}

    def inp(name, shape, dt=MM_DT):
        io[name] = nc.dram_tensor(name, shape, dt, kind="ExternalInput").ap()

    for nm in ("r", "i", "s"):
        inp(f"x_{nm}_0", [128, 4 * S])
        inp(f"x_{nm}_1", [128, 4 * S])
        inp(f"wq_{nm}", [2 * NK, 128, NK * 128])
        inp(f"wv_{nm}", [2, 128, NK * 512])
        inp(f"wo_{nm}", [2, 128, NK * 512])
    inp("cos", [128, S], TAB_DT)
    inp("sin", [128, S], TAB_DT)
    inp("cos_q", [128, S], TAB_DT)
    inp("sin_q", [128, S], TAB_DT)
    io["out"] = nc.dram_tensor("out", [2, S, DM], MM_DT,
                               kind="ExternalOutput").ap()

    with tile.TileContext(nc) as tc:
        build_body(nc, tc, io)
    nc.compile()
    return nc


def host_inputs(xr, xi, wqkv_r, wqkv_i, wo_r, wo_i):
    """Pack full f32 inputs into 8 per-core in_maps."""
    np_mm = mybir.dt.np(MM_DT)
    np_tab = mybir.dt.np(TAB_DT)

    def pack_qk(w):  # (D, 3D) -> [16e][128p][8k*128]
        return np.ascontiguousarray(
            w[:, :2 * DM].reshape(NK, 128, 2 * NK, 128).transpose(2, 1, 0, 3)
            .reshape(2 * NK, 128, NK * 128))

    def pack_v(w):  # -> [2n][128p][8k*512]
        return np.ascontiguousarray(
            w[:, 2 * DM:].reshape(NK, 128, 2, 512).transpose(2, 1, 0, 3)
            .reshape(2, 128, NK * 512))

    def pack_p(w):  # (NK,128,F) row-major -> [128p][NK*F]
        return np.ascontiguousarray(
            w.transpose(1, 0, 2).reshape(128, -1))

    def pack_wo(w):  # (D=e, D=out) -> [2n][128p][NK*512]
        return np.ascontiguousarray(
            w.reshape(NK, 128, 2, 512).transpose(2, 1, 0, 3)
            .reshape(2, 128, NK * 512))

    wqkvT_r = np.ascontiguousarray(wqkv_r.T).astype(np_mm)  # (D, 3D)
    wqkvT_i = np.ascontiguousarray(wqkv_i.T).astype(np_mm)
    wqkvT_s = (wqkvT_r.astype(np.float32)
               + wqkvT_i.astype(np.float32)).astype(np_mm)
    woT_r = np.ascontiguousarray(wo_r.T.astype(np_mm))  # (D_in=e, D_out)
    woT_i = np.ascontiguousarray(wo_i.T.astype(np_mm))
    woT_s = (woT_r.astype(np.float32)
             + woT_i.astype(np.float32)).astype(np_mm)
    # o_i e-rows come out of the AV pass pair-swapped: permute wo_i rows to
    # match ([h1|h0] within each pair)
    perm = np.arange(DM).reshape(NP, 2, DH)[:, ::-1, :].reshape(DM)
    woT_i = np.ascontiguousarray(woT_i[perm])

    inv_freq = 1.0 / (10000.0 ** (np.arange(DH, dtype=np.float64) / DH))
    ang = np.arange(S, dtype=np.float64)[:, None] * inv_freq[None, :]  # (S,Dh)
    cosT = np.cos(ang).T  # (Dh, S)
    sinT = np.sin(ang).T

    def dup(t):
        return np.ascontiguousarray(np.concatenate([t, t], axis=0))

    shared = {
        "wq_r": pack_qk(wqkvT_r), "wq_i": pack_qk(wqkvT_i),
        "wq_s": pack_qk(-wqkvT_i.astype(np.float32)).astype(np_mm),
        "wv_r": pack_v(wqkvT_r), "wv_i": pack_v(wqkvT_i),
        "wv_s": pack_v(wqkvT_s),
        "wo_r": pack_wo(woT_r), "wo_i": pack_wo(woT_i),
        "wo_s": pack_wo(woT_s),
        "cos": dup(cosT).astype(np_tab), "sin": dup(sinT).astype(np_tab),
        "cos_q": (dup(cosT) * SCALE).astype(np_tab),
        "sin_q": (dup(sinT) * SCALE).astype(np_tab),
    }
    in_maps = []
    for b in range(B):
        xT_r = xr[b].T.astype(np_mm).reshape(NK, 128, S)
        xT_i = xi[b].T.astype(np_mm).reshape(NK, 128, S)
        xT_s = (xT_r.astype(np.float32)
                + xT_i.astype(np.float32)).astype(np_mm)
        m = {}
        for nm, t in (("r", xT_r), ("i", xT_i), ("s", xT_s)):
            packed = pack_p(t)  # [128, NK*S]
            m[f"x_{nm}_0"] = np.ascontiguousarray(packed[:, :4 * S])
            m[f"x_{nm}_1"] = np.ascontiguousarray(packed[:, 4 * S:])
        m.update(shared)
        in_maps.append(m)
    return in_maps


_NC_CACHE = None


def get_nc():
    global _NC_CACHE
    if _NC_CACHE is None:
        _NC_CACHE = build_nc()
    return _NC_CACHE


def kernel(xr, xi, wqkv_r, wqkv_i, wo_r, wo_i):
    from concourse.bass_utils import run_bass_kernel_spmd

    _install_act_root()
    in_maps = host_inputs(np.asarray(xr, np.float32),
                          np.asarray(xi, np.float32),
                          np.asarray(wqkv_r, np.float32),
                          np.asarray(wqkv_i, np.float32),
                          np.asarray(wo_r, np.float32),
                          np.asarray(wo_i, np.float32))
    nc = get_nc()
    res = run_bass_kernel_spmd(nc, in_maps, core_ids=list(range(B)),
                               trace=bool(int(os.environ.get("K_TRACE", "0"))))
    out_r = np.stack([res.results[b]["out"][0].astype(np.float32)
                      for b in range(B)])
    out_i = np.stack([res.results[b]["out"][1].astype(np.float32)
                      for b in range(B)])
    kernel.last_results = res
    return out_r, out_i


# revision 34
# speedup vs baseline: 1.0028x; 1.0028x over previous
"""CartesianDecomposedAttention Trainium2 kernel (v2).

Complex-valued MHA (B=8, S=512, D=1024, H=16, Dh=64) decomposed into real
arithmetic, data-parallel over batch across 8 NeuronCores (one batch element
per core, no collectives).

Key structure (v2, rebuilt from the v1 baseline's trace analysis):
  - QKV/WO projections: 3-matmul Karatsuba complex products; combines are
    DVE subs reading PSUM f32 directly and writing bf16 (rounding AFTER the
    subtraction - more accurate and cheaper than staging via ACT copies).
  - RoPE writes q/k tiles in a dh-STACKED layout: Mq_h=[qr_h;qi_h],
    Sre_h=[kr_h;ki_h], Sim_h=[-ki_h;kr_h] so the scores matmuls contract
    over the full 128 partitions (one pass per Re/Im per head per t-tile,
    half the PE passes of the 64-contraction variant).
  - Softmax: exp comes straight off the ACT Exp table (PSUM in, bf16 out);
    the act-root is restricted to {exp_and_others, trig_and_small} and ACT
    ops are grouped so there are exactly 2 table loads per head pair.
    Phase rotors: ADD_RANGE_WRAP into [-pi,pi], sin via table,
    cos = sin(pi/2 - |x|).
  - Softmax denominator: ones[128,128]-stationary matmul accumulating over
    t-tiles gives D[s] broadcast across all 128 PSUM rows; one DVE
    reciprocal produces a full-width rb tile (no partition broadcasts).
  - AV: stationary packs [vr|vi] / [vn|vr] per head so or/oi accumulate in
    ONE psum per head (2 passes per (head, t-tile) instead of 4); the o_i
    e-rows come out pair-swapped, compensated by a host-side row
    permutation of wo_i.
  - Output written bf16 (host upcasts), halving the tail DMA.

Matmul operands bf16 (fp32 PSUM accumulation); softmax trig fp32.
"""

import os
import sys

sys.path.insert(0, "/opt/trn_rl_repo")

import math

import ml_dtypes
import numpy as np

import concourse.bass as bass
import concourse.mybir as mybir
import concourse.tile as tile
from concourse import bacc
from concourse.dve_ops import ADD_RANGE_WRAP
from bass_rust import add_dep_helper

BF16 = ml_dtypes.bfloat16

B, S, DM, H, DH = 8, 512, 1024, 16, 64
NK = DM // 128          # 8 contraction chunks of 128
NP = H // 2             # 8 head pairs
NST = S // 128          # 4 t-tiles
SCALE = 1.0 / math.sqrt(DH)
TWO_PI = 2.0 * math.pi
SSC = 1.0 - 1e-6

MM_DT = mybir.dt.bfloat16
TAB_DT = mybir.dt.bfloat16
F32 = mybir.dt.float32


def build_body(nc, tc, io):
    AF = mybir.ActivationFunctionType
    ALU = mybir.AluOpType
    V = nc.vector
    G = nc.gpsimd
    A = nc.scalar
    PE = nc.tensor

    out = io["out"]

    const = tc.alloc_tile_pool(name="const", bufs=1)
    ps = tc.alloc_tile_pool(name="ps", bufs=3, space="PSUM")
    psc = tc.alloc_tile_pool(name="psc", bufs=3, space="PSUM")
    pav = tc.alloc_tile_pool(name="pav", bufs=2, space="PSUM")
    qk_pool = tc.alloc_tile_pool(name="qk", bufs=1)
    v_pool = tc.alloc_tile_pool(name="vp", bufs=1)
    o_pool = tc.alloc_tile_pool(name="op", bufs=1)
    work = tc.alloc_tile_pool(name="wk", bufs=1)
    pool_x = tc.alloc_tile_pool(name="pool_x", bufs=1)

    # ---------------- prologue DMAs (priority order) ----------------
    # pair-0 q-etile weights first (unblocks the very first matmul), x
    # pieces split in half per stream so chunk-0 matmuls start early.
    wq_cur = {}

    def load_wq(which, j, eng=None):
        et = j if which == "q" else NK + j
        for nm in ("r", "i", "s"):
            t = pool_x.tile([128, NK * 128], MM_DT, name=f"w_{nm}_{which}{j}",
                          tag=f"wst_{nm}", bufs=2)
            (eng or nc.sync).dma_start(t[:], io[f"wq_{nm}"][et])
            wq_cur[(which, nm)] = t

    load_wq("q", 0)
    x_sb = {"r": [None, None], "i": [None, None], "s": [None, None]}
    for nm, piece, eng in (("r", 0, nc.scalar), ("i", 0, nc.gpsimd),
                           ("r", 1, nc.scalar), ("i", 1, nc.gpsimd)):
        t = pool_x.tile([128, 4 * S], MM_DT, name=f"x_{nm}_{piece}",
                        tag=f"x{nm}{piece}", bufs=1)
        eng.dma_start(t[:], io[f"x_{nm}_{piece}"][:])
        x_sb[nm][piece] = t
    load_wq("k", 0, eng=nc.scalar)

    # ---------------- constants ----------------
    cos_sb = const.tile([128, S], TAB_DT, name="cos_sb")
    sin_sb = const.tile([128, S], TAB_DT, name="sin_sb")
    cosq_sb = const.tile([128, S], TAB_DT, name="cosq_sb")
    sinq_sb = const.tile([128, S], TAB_DT, name="sinq_sb")
    nc.gpsimd.dma_start(cosq_sb[:], io["cos_q"][:])
    nc.gpsimd.dma_start(sinq_sb[:], io["sin_q"][:])
    nc.gpsimd.dma_start(cos_sb[:], io["cos"][:])
    nc.gpsimd.dma_start(sin_sb[:], io["sin"][:])
    for piece in range(2):
        t = pool_x.tile([128, 4 * S], MM_DT, name=f"x_s_{piece}",
                        tag=f"xs{piece}", bufs=1)
        nc.gpsimd.dma_start(t[:], io[f"x_s_{piece}"][:])
        x_sb["s"][piece] = t
    ones_sb = const.tile([128, 128], MM_DT, name="ones_sb")
    G.memset(ones_sb[:], 1.0)
    b_hpi = const.tile([128, 1], F32, name="b_hpi")
    G.memset(b_hpi[:], math.pi / 2)

    def xch(nm, k):
        return x_sb[nm][k // 4][:, (k % 4) * S:(k % 4 + 1) * S]

    # ---------------- QK projection + RoPE (stacked layout) -------------
    def qk_pair(j):
        """Emit projection + rope for pair j; returns (Mq, Sre, Sim)."""
        res = {}
        for which in ("q", "k"):
            if not (which == "q" and j == 0):
                if not (which == "k" and j == 0):
                    load_wq(which, j)
            w_r = wq_cur[(which, "r")]
            w_i = wq_cur[(which, "i")]
            w_n = wq_cur[(which, "s")]  # holds -wi (4-mult complex)
            p_ar = ps.tile([128, 512], F32, name=f"qk1_{which}{j}", tag="ps")
            p_ai = ps.tile([128, 512], F32, name=f"qk2_{which}{j}", tag="ps")
            for k in range(NK):
                ksl = slice(k * 128, (k + 1) * 128)
                PE.matmul(p_ar[:], w_r[:, ksl], xch("r", k), start=(k == 0),
                          stop=False)
            for k in range(NK):
                ksl = slice(k * 128, (k + 1) * 128)
                PE.matmul(p_ar[:], w_n[:, ksl], xch("i", k), start=False,
                          stop=(k == NK - 1))
            for k in range(NK):
                ksl = slice(k * 128, (k + 1) * 128)
                PE.matmul(p_ai[:], w_i[:, ksl], xch("r", k), start=(k == 0),
                          stop=False)
            for k in range(NK):
                ksl = slice(k * 128, (k + 1) * 128)
                PE.matmul(p_ai[:], w_r[:, ksl], xch("i", k), start=False,
                          stop=(k == NK - 1))
            c_t = cosq_sb if which == "q" else cos_sb
            s_t = sinq_sb if which == "q" else sin_sb
            t1 = work.tile([128, 512], MM_DT, name=f"t1_{which}{j}", tag="t1",
                           bufs=1)
            t2 = work.tile([128, 512], MM_DT, name=f"t2_{which}{j}", tag="t2",
                           bufs=1)
            t3 = work.tile([128, 512], MM_DT, name=f"t3_{which}{j}", tag="t3",
                           bufs=1)
            t4 = work.tile([128, 512], MM_DT, name=f"t4_{which}{j}", tag="t4",
                           bufs=1)
            V.tensor_mul(t1[:], p_ar[:], c_t[:])
            V.tensor_mul(t2[:], p_ai[:], s_t[:])
            V.tensor_mul(t3[:], p_ar[:], s_t[:])
            V.tensor_mul(t4[:], p_ai[:], c_t[:])
            tiles = []
            for half in range(2):
                hs = slice(half * 64, (half + 1) * 64)
                if which == "q":
                    mq = qk_pool.tile([128, 512], MM_DT,
                                      name=f"mq_{j}_{half}", tag="mq", bufs=5)
                    V.tensor_sub(mq[0:64, :], t1[hs, :], t2[hs, :])
                    V.tensor_add(mq[64:128, :], t3[hs, :], t4[hs, :])
                    tiles.append(mq)
                else:
                    sre = qk_pool.tile([128, 512], MM_DT,
                                       name=f"sre_{j}_{half}", tag="sre",
                                       bufs=5)
                    sim = qk_pool.tile([128, 512], MM_DT,
                                       name=f"sim_{j}_{half}", tag="sim",
                                       bufs=5)
                    V.tensor_sub(sre[0:64, :], t1[hs, :], t2[hs, :])
                    V.tensor_add(sre[64:128, :], t3[hs, :], t4[hs, :])
                    V.tensor_scalar_mul(sim[0:64, :], sre[64:128, :], -1.0)
                    V.tensor_scalar_mul(sim[64:128, :], sre[0:64, :], 1.0)
                    tiles.append((sre, sim))
            res[which] = tiles
        return res["q"], [t[0] for t in res["k"]], [t[1] for t in res["k"]]

    # ---------------- V projection into packed AV layouts ----------------
    # v_AB[tt] blocks per head h: even h -> [vr|vi], odd h -> [vi|vr]
    # v_CD[tt] blocks per head h: even h -> [vn|vr], odd h -> [vr|vn]
    v_AB = [v_pool.tile([128, 2048], MM_DT, name=f"vab_{tt}", tag="vab",
                        bufs=NST) for tt in range(NST)]
    v_CD = [v_pool.tile([128, 2048], MM_DT, name=f"vcd_{tt}", tag="vcd",
                        bufs=NST) for tt in range(NST)]

    wv_cur = {}

    def load_wv(n):
        for nm, eng in (("r", nc.gpsimd), ("i", nc.gpsimd), ("s", nc.scalar)):
            t = pool_x.tile([128, NK * 512], MM_DT, name=f"wv_{nm}_{n}",
                          tag=f"wv{nm}", bufs=1)
            eng.dma_start(t[:], io[f"wv_{nm}"][n])
            wv_cur[nm] = t

    def quad4(ap1024, q):
        # [128,1024] -> [128,4,64] selecting 64-col blocks at base q*64,
        # stride 256
        return ap1024.rearrange("p (k q i) -> p k q i", k=4, q=4, i=64)[
            :, :, q, :]

    def half8(ap512, q):
        # psum [128,512] -> [128,4,64] selecting head-blocks at base q*64,
        # stride 128 (q=0: even heads, q=1: odd heads)
        return ap512.rearrange("p (k q i) -> p k q i", k=4, q=2, i=64)[
            :, :, q, :]

    def v_chunk(n, st, pool=None):
        pool = pool or ps
        tag = "ps" if pool is ps else "psc"

        def wvch(nm, k):
            return wv_cur[nm][:, k * 512:(k + 1) * 512]

        ssl = slice(st * 128, (st + 1) * 128)
        p1 = pool.tile([128, 512], F32, name=f"vp1_{st}_{n}", tag=tag)
        p2 = pool.tile([128, 512], F32, name=f"vp2_{st}_{n}", tag=tag)
        p3 = pool.tile([128, 512], F32, name=f"vp3_{st}_{n}", tag=tag)
        for k in range(NK):
            PE.matmul(p1[:], xch("r", k)[:, ssl], wvch("r", k),
                      start=(k == 0), stop=(k == NK - 1))
        for k in range(NK):
            PE.matmul(p2[:], xch("i", k)[:, ssl], wvch("i", k),
                      start=(k == 0), stop=(k == NK - 1))
        for k in range(NK):
            PE.matmul(p3[:], xch("s", k)[:, ssl], wvch("s", k),
                      start=(k == 0), stop=(k == NK - 1))
        ab = v_AB[st][:, n * 1024:(n + 1) * 1024]
        cd = v_CD[st][:, n * 1024:(n + 1) * 1024]
        pc = work.tile([128, 512], F32, name=f"vpc_{st}_{n}", tag="stg",
                       bufs=2)
        V.tensor_scalar_mul(pc[:], p1[:], 1.0)
        # vr = p1 - p2 into AB (even:+0, odd:+64)
        V.tensor_sub(quad4(ab, 0), half8(pc[:], 0), half8(p2[:], 0))
        V.tensor_sub(quad4(ab, 3), half8(pc[:], 1), half8(p2[:], 1))
        # vi = p3 - p1 - p2 into AB (even:+64, odd:+0)
        tf = work.tile([128, 512], F32, name=f"vtf_{st}_{n}", tag="tf",
                       bufs=2)
        V.tensor_sub(tf[:], p3[:], pc[:])
        V.tensor_sub(quad4(ab, 1), half8(tf[:], 0), half8(p2[:], 0))
        V.tensor_sub(quad4(ab, 2), half8(tf[:], 1), half8(p2[:], 1))
        # vr copies into CD (even:+64, odd:+0)
        V.tensor_scalar_mul(quad4(cd, 1), quad4(ab, 0), 1.0)
        V.tensor_scalar_mul(quad4(cd, 2), quad4(ab, 3), 1.0)
        # vn = -vi into CD (even:+0, odd:+64)
        V.tensor_scalar_mul(quad4(cd, 0), quad4(ab, 1), -1.0)
        V.tensor_scalar_mul(quad4(cd, 3), quad4(ab, 2), -1.0)

    # ---------------- output accumulators ----------------
    o_r = [o_pool.tile([128, S], MM_DT, name=f"o_r_{j}", tag="o_r", bufs=NP)
           for j in range(NP)]
    o_i = [o_pool.tile([128, S], MM_DT, name=f"o_i_{j}", tag="o_i", bufs=NP)
           for j in range(NP)]
    o_s = [o_pool.tile([128, S], MM_DT, name=f"o_s_{j}", tag="o_s", bufs=NP)
           for j in range(NP)]

    # ---------------- scores + softmax + AV ----------------
    av_state = {}

    def scores_and_av(j, qk, prev):
        """Emit pair j's scores/softmax; interleave pair prev's denominator
        and AV matmuls into the same PE window."""
        Mq, Sre, Sim = qk
        ebs = [work.tile([128, NST * 512], MM_DT, name=f"eb_{j}_{h}",
                         tag="eb", bufs=2) for h in range(2)]
        phs = [work.tile([128, 2048], F32, name=f"ph_{j}_{h}", tag="ph",
                         bufs=2) for h in range(2)]

        rb_prev = None
        if prev is not None:
            rb_prev = av_state[prev][3]
            ps_o = [pav.tile([128, 512], F32, name=f"av_{prev}_{h}",
                             tag="pav") for h in range(2)]
            Wr_p, Wi_p = av_state[prev][1], av_state[prev][2]

        # scores matmuls (+ AV(prev) interleaved per tt)
        for tt in range(NST):
            tsl = slice(tt * 128, (tt + 1) * 128)
            csl = slice(tt * 512, (tt + 1) * 512)
            for half in range(2):
                ps_re = psc.tile([128, 512], F32, name=f"re_{j}_{half}_{tt}",
                                 tag="psc")
                ps_im = psc.tile([128, 512], F32, name=f"im_{j}_{half}_{tt}",
                                 tag="psc")
                PE.matmul(ps_re[:], Sre[half][:, tsl], Mq[half][:],
                          start=True, stop=True)
                PE.matmul(ps_im[:], Sim[half][:, tsl], Mq[half][:],
                          start=True, stop=True)
                A.activation(ebs[half][:, csl], ps_re[:], AF.Exp,
                             scale=1.0)
                V._custom_dve(ADD_RANGE_WRAP,
                              out=phs[half][:, tt * 512:(tt + 1) * 512],
                              in0=ps_im[:], s0=0.0, s1=math.pi, imm2=TWO_PI)
            if prev is not None:
                h2 = 2 * prev
                for half in range(2):
                    blk = slice((h2 + half) * 128, (h2 + half + 1) * 128)
                    PE.matmul(ps_o[half][:], v_AB[tt][:, blk],
                              Wr_p[half][:, csl], start=(tt == 0), stop=False,
                              skip_group_check=True)
                    PE.matmul(ps_o[half][:], v_CD[tt][:, blk],
                              Wi_p[half][:, csl], start=False,
                              stop=(tt == NST - 1), skip_group_check=True)

        # denominator for THIS pair right after its exps (fills the PE
        # window before trig completes; recip gets a full iteration of lead)
        rb_self = []
        for half in range(2):
            ps_d = psc.tile([128, 512], F32, name=f"dn_{j}_{half}",
                            tag="psc")
            for tt in range(NST):
                PE.matmul(ps_d[:], ones_sb[:],
                          ebs[half][:, tt * 512:(tt + 1) * 512],
                          start=(tt == 0), stop=(tt == NST - 1))
            rb = work.tile([128, 512], F32, name=f"rb_{j}_{half}",
                           tag="rb", bufs=3)
            V.reciprocal_approx_fast(out=rb[:], in_=ps_d[:])
            rb_self.append(rb)
        # trig (single table switch per pair: all exps above, sins below).
        # A value-preserving mark on each ph tile makes every sin's input
        # depend on the pair's last exps, so the scheduler can't interleave
        # exps and sins (would thrash ACT table loads).
        if j < NP - 1:
            tok = work.tile([128, 1], F32, name=f"tok_{j}", tag="tok",
                            bufs=2)
            V.scalar_tensor_tensor(tok[:], ebs[0][:, 2047:2048], 0.0,
                                   ebs[1][:, 2047:2048], ALU.mult, ALU.add)
            for half in range(2):
                for c in range(2):
                    cc = c * 1024
                    V.scalar_tensor_tensor(phs[half][:, cc:cc + 1], tok[:],
                                           0.0, phs[half][:, cc:cc + 1],
                                           ALU.mult, ALU.add)
        s1 = [[None] * 2 for _ in range(2)]
        c1 = [[None] * 2 for _ in range(2)]
        for half in range(2):
            for c in range(2):
                s1t = work.tile([128, 1024], MM_DT, name=f"s1_{j}_{half}_{c}",
                                tag="s1", bufs=2)
                c1t = work.tile([128, 1024], MM_DT, name=f"c1_{j}_{half}_{c}",
                                tag="c1", bufs=2)
                ph = phs[half][:, c * 1024:(c + 1) * 1024]
                A.activation(s1t[:], ph, AF.Sin, scale=SSC)
                A.activation(ph, ph, AF.Abs, scale=1.0)
                A.activation(c1t[:], ph, AF.Sin, bias=b_hpi[:], scale=-1.0)
                s1[half][c] = s1t
                c1[half][c] = c1t
        # W~ = e * (cos, sin); Wr on gpsimd to offload the DVE
        Wr = [work.tile([128, NST * 512], MM_DT, name=f"Wr_{j}_{h}", tag="Wr",
                        bufs=2) for h in range(2)]
        Wi = [work.tile([128, NST * 512], MM_DT, name=f"Wi_{j}_{h}", tag="Wi",
                        bufs=2) for h in range(2)]
        for half in range(2):
            for c in range(2):
                cl = slice(c * 1024, (c + 1) * 1024)
                V.tensor_mul(Wr[half][:, cl], ebs[half][:, cl],
                             c1[half][c][:])
                V.tensor_mul(Wi[half][:, cl], ebs[half][:, cl], s1[half][c][:])

        # AV(prev) eviction
        if prev is not None:
            pj = prev
            V.tensor_mul(o_r[pj][0:64, :], ps_o[0][0:64, :], rb_prev[0][0:64, :])
            V.tensor_mul(o_r[pj][64:128, :], ps_o[1][64:128, :],
                         rb_prev[1][64:128, :])
            V.tensor_mul(o_i[pj][0:64, :], ps_o[1][0:64, :], rb_prev[1][0:64, :])
            V.tensor_mul(o_i[pj][64:128, :], ps_o[0][64:128, :],
                         rb_prev[0][64:128, :])
            osa = work.tile([128, 512], MM_DT, name=f"osa_{pj}", tag="osa",
                            bufs=1)
            V.tensor_scalar_mul(osa[0:64, :], o_i[pj][64:128, :], 1.0)
            V.tensor_scalar_mul(osa[64:128, :], o_i[pj][0:64, :], 1.0)
            V.tensor_add(o_s[pj][:, :], o_r[pj][:, :], osa[:, :])

        av_state[j] = (ebs, Wr, Wi, rb_self)
        if prev is not None:
            del av_state[prev]

    # ---------------- phase 3: output projection (Karatsuba) -------------
    wo_sb = {}

    def load_wo_n0():
        # n=0 halves reuse the wv tag slots (all wv reads done by v(1,3))
        for nm, eng, tag in (("r", nc.sync, "wvr"), ("i", nc.scalar, "wvi"),
                             ("s", nc.gpsimd, "wvs")):
            t = pool_x.tile([128, NK * 512], MM_DT, name=f"wo_{nm}_0",
                            tag=tag, bufs=1)
            eng.dma_start(t[:], io[f"wo_{nm}"][0])
            wo_sb[(nm, 0)] = t

    def load_wo_n1():
        # n=1 halves reuse the x tag slots (x reads done by qk_pair(7))
        slots = {"r": ("xs0", "xs1"), "i": ("xi0", "xi1"),
                 "s": ("xr0", "xr1")}
        for nm, eng in (("r", nc.sync), ("i", nc.gpsimd), ("s", nc.scalar)):
            halves = []
            for piece in range(2):
                t = pool_x.tile([128, 4 * 512], MM_DT,
                                name=f"wo_{nm}_1_{piece}",
                                tag=slots[nm][piece], bufs=1)
                eng.dma_start(t[:], io[f"wo_{nm}"][1][:, piece * 2048:
                                                     (piece + 1) * 2048])
                halves.append(t)
            wo_sb[(nm, 1)] = halves

    def woch(nm, k, n):
        if n == 0:
            return wo_sb[(nm, 0)][:, k * 512:(k + 1) * 512]
        t = wo_sb[(nm, 1)][k // 4]
        kk = k % 4
        return t[:, kk * 512:(kk + 1) * 512]

    def phase3_unit(st, n, pool, upto=NK):
        ssl = slice(st * 128, (st + 1) * 128)
        tag = "ps" if pool is ps else "psc"
        p1 = pool.tile([128, 512], F32, name=f"pj1_{st}_{n}", tag=tag)
        p2 = pool.tile([128, 512], F32, name=f"pj2_{st}_{n}", tag=tag)
        p3 = pool.tile([128, 512], F32, name=f"pj3_{st}_{n}", tag=tag)
        state = {"k": 0}

        def advance(upto2):
            for k in range(state["k"], upto2):
                PE.matmul(p1[:], o_r[k][:, ssl], woch("r", k, n),
                          start=(k == 0), stop=(k == NK - 1))
                PE.matmul(p2[:], o_i[k][:, ssl], woch("i", k, n),
                          start=(k == 0), stop=(k == NK - 1))
                PE.matmul(p3[:], o_s[k][:, ssl], woch("s", k, n),
                          start=(k == 0), stop=(k == NK - 1))
            state["k"] = upto2

        def finish():
            advance(NK)
            to_r = work.tile([128, 512], MM_DT, name=f"otr_{st}_{n}",
                             tag="out_r", bufs=1)
            to_i = work.tile([128, 512], MM_DT, name=f"oti_{st}_{n}",
                             tag="out_i", bufs=1)
            tf3 = work.tile([128, 512], F32, name=f"otf_{st}_{n}",
                            tag="out_f", bufs=1)
            pc3 = work.tile([128, 512], F32, name=f"opc_{st}_{n}",
                            tag="stg", bufs=2)
            V.tensor_scalar_mul(pc3[:], p1[:], 1.0)
            V.tensor_sub(to_r[:], pc3[:], p2[:])
            V.tensor_sub(tf3[:], p3[:], pc3[:])
            V.tensor_sub(to_i[:], tf3[:], p2[:])
            nsl = slice(n * 512, (n + 1) * 512)
            nc.sync.dma_start(out[0, ssl, nsl], to_r[:])
            nc.sync.dma_start(out[1, ssl, nsl], to_i[:])

        advance(upto)
        return advance, finish

    # ---------------- emission schedule ----------------
    load_wv(0)
    qk0 = qk_pair(0)
    qk1 = qk_pair(1)
    scores_and_av(0, qk0, None)
    for st in range(NST):
        v_chunk(0, st)
    qk_tiles = {1: qk1}
    pre_units = []
    for j in range(1, NP):
        if j + 1 < NP:
            qk_tiles[j + 1] = qk_pair(j + 1)
            if j + 1 == NP - 1:
                load_wo_n1()
        if j == 5:
            load_wo_n0()
        if j == NP - 1:
            # fill the last pair's softmax latency with phase-3 partials
            # (ps pool only - psc is still needed by scores/denom)
            pre_units.append(phase3_unit(0, 0, ps, upto=NK - 2))
        scores_and_av(j, qk_tiles.pop(j), j - 1)
        if j == 1:
            load_wv(1)
        if 1 <= j <= NST:
            v_chunk(1, j - 1)

    # last pair's denominator+AV, interleaved with the first phase-3 units
    for adv, _f in pre_units:
        adv(NK - 1)
    # emit AV for pair 7 (denominator already computed in its scores pass)
    prev = NP - 1
    rb_prev = av_state[prev][3]
    pre_units.append(phase3_unit(1, 0, psc, upto=NK - 1))
    ps_o = [pav.tile([128, 512], F32, name=f"av_{prev}_{h}", tag="pav")
            for h in range(2)]
    Wr_p, Wi_p = av_state[prev][1], av_state[prev][2]
    h2 = 2 * prev
    for tt in range(NST):
        csl = slice(tt * 512, (tt + 1) * 512)
        for half in range(2):
            blk = slice((h2 + half) * 128, (h2 + half + 1) * 128)
            PE.matmul(ps_o[half][:], v_AB[tt][:, blk], Wr_p[half][:, csl],
                      start=(tt == 0), stop=False, skip_group_check=True)
            PE.matmul(ps_o[half][:], v_CD[tt][:, blk], Wi_p[half][:, csl],
                      start=False, stop=(tt == NST - 1),
                      skip_group_check=True)
    pj = prev
    V.tensor_mul(o_r[pj][0:64, :], ps_o[0][0:64, :], rb_prev[0][0:64, :])
    V.tensor_mul(o_r[pj][64:128, :], ps_o[1][64:128, :], rb_prev[1][64:128, :])
    V.tensor_mul(o_i[pj][0:64, :], ps_o[1][0:64, :], rb_prev[1][0:64, :])
    V.tensor_mul(o_i[pj][64:128, :], ps_o[0][64:128, :], rb_prev[0][64:128, :])
    osa = work.tile([128, 512], MM_DT, name=f"osa_{pj}", tag="osa", bufs=1)
    V.tensor_scalar_mul(osa[0:64, :], o_i[pj][64:128, :], 1.0)
    V.tensor_scalar_mul(osa[64:128, :], o_i[pj][0:64, :], 1.0)
    V.tensor_add(o_s[pj][:, :], o_r[pj][:, :], osa[:, :])
    del av_state[prev]

    for _adv, fin in pre_units:
        fin()
    pools3 = [ps, psc]
    i3 = 0
    for n in range(2):
        for st in range(NST):
            if st <= 1 and n == 0:
                continue
            _a, fin = phase3_unit(st, n, pools3[i3 % 2])
            fin()
            i3 += 1

    for p in (pool_x, work, o_pool, v_pool, qk_pool, pav, psc, ps, const):
        p.release()


def _install_act_root():
    """Restrict walrus to the {exp_and_others, trig_and_small} ACT table
    sets so exp and sin are each one load away and nothing else thrashes.
    On any failure, degrade to the default tables (correct, slower)."""
    if os.environ.get("K_NO_ACTFIX"):
        return
    if os.environ.get("BASS_ACT_ROOT_JSON_PATH"):
        return
    try:
        _install_act_root_impl()
    except Exception:
        os.environ["K_NO_ACTFIX"] = "1"


_KEEP_SETS = ("exp_and_others", "trig_and_small")


def _install_act_root_impl():
    import json
    import tempfile
    from neuronxcc.driver.Job import Job
    from neuronxcc.driver.jobs.support.FindActInfo import findActInfoFile

    p = findActInfoFile(Job.getPackageDir(), "gen3")
    src_dir = os.path.dirname(p)
    with open(p) as f:
        d = json.load(f)
    d["act_func_sets"] = [e for e in d["act_func_sets"]
                          if e["name"] in _KEEP_SETS]
    out_dir = tempfile.mkdtemp(prefix="act_expsin_")
    for fn in os.listdir(src_dir):
        sp = os.path.join(src_dir, fn)
        if os.path.isfile(sp) and fn != "act_info.json":
            os.symlink(sp, os.path.join(out_dir, fn))
    with open(os.path.join(out_dir, "act_info.json"), "w") as f:
        json.dump(d, f)
    os.environ["BASS_ACT_ROOT_JSON_PATH"] = os.path.join(out_dir,
                                                         "act_info.json")
    import concourse.hw_specs as hw_specs
    import concourse.bacc as bacc_mod

    orig = hw_specs.get_activation_tables.__wrapped__

    @__import__("functools").cache
    def only_kept(arch):
        full = orig(arch)
        return {k: full[k] for k in _KEEP_SETS}

    hw_specs.get_activation_tables = only_kept
    bacc_mod.get_activation_tables = only_kept


def build_nc():
    _install_act_root()
    nc = bacc.Bacc("TRN2", target_bir_lowering=False, debug=False,
                   enable_asserts=False, num_devices=8)
    io = {}

    def inp(name, shape, dt=MM_DT):
        io[name] = nc.dram_tensor(name, shape, dt, kind="ExternalInput").ap()

    for nm in ("r", "i", "s"):
        inp(f"x_{nm}_0", [128, 4 * S])
        inp(f"x_{nm}_1", [128, 4 * S])
        inp(f"wq_{nm}", [2 * NK, 128, NK * 128])
        inp(f"wv_{nm}", [2, 128, NK * 512])
        inp(f"wo_{nm}", [2, 128, NK * 512])
    inp("cos", [128, S], TAB_DT)
    inp("sin", [128, S], TAB_DT)
    inp("cos_q", [128, S], TAB_DT)
    inp("sin_q", [128, S], TAB_DT)
    io["out"] = nc.dram_tensor("out", [2, S, DM], MM_DT,
                               kind="ExternalOutput").ap()

    with tile.TileContext(nc) as tc:
        build_body(nc, tc, io)
    nc.compile()
    return nc


def host_inputs(xr, xi, wqkv_r, wqkv_i, wo_r, wo_i):
    """Pack full f32 inputs into 8 per-core in_maps."""
    np_mm = mybir.dt.np(MM_DT)
    np_tab = mybir.dt.np(TAB_DT)

    def pack_qk(w):  # (D, 3D) -> [16e][128p][8k*128]
        return np.ascontiguousarray(
            w[:, :2 * DM].reshape(NK, 128, 2 * NK, 128).transpose(2, 1, 0, 3)
            .reshape(2 * NK, 128, NK * 128))

    def pack_v(w):  # -> [2n][128p][8k*512]
        return np.ascontiguousarray(
            w[:, 2 * DM:].reshape(NK, 128, 2, 512).transpose(2, 1, 0, 3)
            .reshape(2, 128, NK * 512))

    def pack_p(w):  # (NK,128,F) row-major -> [128p][NK*F]
        return np.ascontiguousarray(
            w.transpose(1, 0, 2).reshape(128, -1))

    def pack_wo(w):  # (D=e, D=out) -> [2n][128p][NK*512]
        return np.ascontiguousarray(
            w.reshape(NK, 128, 2, 512).transpose(2, 1, 0, 3)
            .reshape(2, 128, NK * 512))

    wqkvT_r = np.ascontiguousarray(wqkv_r.T).astype(np_mm)  # (D, 3D)
    wqkvT_i = np.ascontiguousarray(wqkv_i.T).astype(np_mm)
    wqkvT_s = (wqkvT_r.astype(np.float32)
               + wqkvT_i.astype(np.float32)).astype(np_mm)
    woT_r = np.ascontiguousarray(wo_r.T.astype(np_mm))  # (D_in=e, D_out)
    woT_i = np.ascontiguousarray(wo_i.T.astype(np_mm))
    woT_s = (woT_r.astype(np.float32)
             + woT_i.astype(np.float32)).astype(np_mm)
    # o_i e-rows come out of the AV pass pair-swapped: permute wo_i rows to
    # match ([h1|h0] within each pair)
    perm = np.arange(DM).reshape(NP, 2, DH)[:, ::-1, :].reshape(DM)
    woT_i = np.ascontiguousarray(woT_i[perm])

    inv_freq = 1.0 / (10000.0 ** (np.arange(DH, dtype=np.float64) / DH))
    ang = np.arange(S, dtype=np.float64)[:, None] * inv_freq[None, :]  # (S,Dh)
    cosT = np.cos(ang).T  # (Dh, S)
    sinT = np.sin(ang).T

    def dup(t):
        return np.ascontiguousarray(np.concatenate([t, t], axis=0))

    shared = {
        "wq_r": pack_qk(wqkvT_r), "wq_i": pack_qk(wqkvT_i),
        "wq_s": pack_qk(-wqkvT_i.astype(np.float32)).astype(np_mm),
        "wv_r": pack_v(wqkvT_r), "wv_i": pack_v(wqkvT_i),
        "wv_s": pack_v(wqkvT_s),
        "wo_r": pack_wo(woT_r), "wo_i": pack_wo(woT_i),
        "wo_s": pack_wo(woT_s),
        "cos": dup(cosT).astype(np_tab), "sin": dup(sinT).astype(np_tab),
        "cos_q": (dup(cosT) * SCALE).astype(np_tab),
        "sin_q": (dup(sinT) * SCALE).astype(np_tab),
    }
    in_maps = []
    for b in range(B):
        xT_r = xr[b].T.astype(np_mm).reshape(NK, 128, S)
        xT_i = xi[b].T.astype(np_mm).reshape(NK, 128, S)
        xT_s = (xT_r.astype(np.float32)
                + xT_i.astype(np.float32)).astype(np_mm)
        m = {}
        for nm, t in (("r", xT_r), ("i", xT_i), ("s", xT_s)):
            packed = pack_p(t)  # [128, NK*S]
            m[f"x_{nm}_0"] = np.ascontiguousarray(packed[:, :4 * S])
            m[f"x_{nm}_1"] = np.ascontiguousarray(packed[:, 4 * S:])
        m.update(shared)
        in_maps.append(m)
    return in_maps


_NC_CACHE = None


def get_nc():
    global _NC_CACHE
    if _NC_CACHE is None:
        _NC_CACHE = build_nc()
    return _NC_CACHE


def kernel(xr, xi, wqkv_r, wqkv_i, wo_r, wo_i):
    from concourse.bass_utils import run_bass_kernel_spmd

    _install_act_root()
    in_maps = host_inputs(np.asarray(xr, np.float32),
                          np.asarray(xi, np.float32),
                          np.asarray(wqkv_r, np.float32),
                          np.asarray(wqkv_i, np.float32),
                          np.asarray(wo_r, np.float32),
                          np.asarray(wo_i, np.float32))
    nc = get_nc()
    res = run_bass_kernel_spmd(nc, in_maps, core_ids=list(range(B)),
                               trace=bool(int(os.environ.get("K_TRACE", "0"))))
    out_r = np.stack([res.results[b]["out"][0].astype(np.float32)
                      for b in range(B)])
    out_i = np.stack([res.results[b]["out"][1].astype(np.float32)
                      for b in range(B)])
    kernel.last_results = res
    return out_r, out_i


# revision 35
# speedup vs baseline: 1.0035x; 1.0007x over previous
"""CartesianDecomposedAttention Trainium2 kernel (v2).

Complex-valued MHA (B=8, S=512, D=1024, H=16, Dh=64) decomposed into real
arithmetic, data-parallel over batch across 8 NeuronCores (one batch element
per core, no collectives).

Key structure (v2, rebuilt from the v1 baseline's trace analysis):
  - Q/K projection: 4-mult complex via negated-weight PSUM accumulation
    (wq_s holds -wi); ar/ai land directly in PSUM, so RoPE multiplies read
    PSUM with no Karatsuba combine stage (2 PSUM banks/etile, not 3).
    V and WO projections stay 3-matmul Karatsuba (combines stage ONE psum
    through a DVE tensor_scalar copy - HW allows only one PSUM operand per
    vector op, and inputs in SBUF must share a base partition).
  - RoPE writes q/k in a dh-STACKED layout: Mq_h=[qr_h;qi_h],
    Sre_h=[kr_h;ki_h], Sim_h=[-ki_h;kr_h], so scores matmuls contract over
    the full 128 partitions (half the PE passes of the 64-contraction
    variant; the PE runs concurrent tile_position matmuls serially, so
    row-splitting buys nothing).
  - Softmax: exp straight off the ACT Exp table (PSUM in, bf16 out). The
    act-root is restricted to {exp_and_others, trig_and_small}; a
    value-preserving DVE mark chains each pair's sins after its exps so the
    list scheduler cannot interleave them (2 ACT table loads per pair).
    Phase rotors: ADD_RANGE_WRAP into [-pi,pi], sin via table,
    cos = sin(pi/2-|x|). The last pair skips the grouping (latency beats
    table loads once there is no later work to hide it).
  - Denominator: ones[128,128]-stationary matmuls accumulate over t-tiles,
    giving D[s] broadcast across all PSUM rows; one reciprocal per half
    yields a full-width rb tile (no partition broadcasts). Emitted with its
    own pair's scores so the reciprocal has a full iteration of lead.
  - AV: stationary packs [vr|vi]/[vn|vr] per head so or/oi accumulate in
    ONE psum per head (2 passes per (head,t-tile) instead of 4); the o_i
    e-rows come out pair-swapped, compensated by a host-side row
    permutation of wo_i. All W~ multiplies stay on the DVE - gpsimd shares
    an SBUF port with it and concurrent pool work stretches DVE ops ~2x.
  - wo streams into the tag slots freed by wv (n=0, free after the last
    v_chunk) and x (n=1, free after the last qk projection), so phase 3
    never waits on a bulk reload; phase-3 units alternate the ps/psc PSUM
    pools and the first two units pre-run k=0..5 under the last pair's
    softmax latency.
  - Output written bf16 (host upcasts), halving the tail DMA.

Matmul operands bf16 (fp32 PSUM accumulation); softmax trig fp32.
"""

import os
import sys

sys.path.insert(0, "/opt/trn_rl_repo")

import math

import ml_dtypes
import numpy as np

import concourse.bass as bass
import concourse.mybir as mybir
import concourse.tile as tile
from concourse import bacc
from concourse.dve_ops import ADD_RANGE_WRAP

BF16 = ml_dtypes.bfloat16

B, S, DM, H, DH = 8, 512, 1024, 16, 64
NK = DM // 128          # 8 contraction chunks of 128
NP = H // 2             # 8 head pairs
NST = S // 128          # 4 t-tiles
SCALE = 1.0 / math.sqrt(DH)
TWO_PI = 2.0 * math.pi
SSC = 1.0 - 1e-6

MM_DT = mybir.dt.bfloat16
TAB_DT = mybir.dt.bfloat16
F32 = mybir.dt.float32


def build_body(nc, tc, io):
    AF = mybir.ActivationFunctionType
    ALU = mybir.AluOpType
    V = nc.vector
    G = nc.gpsimd
    A = nc.scalar
    PE = nc.tensor

    out = io["out"]

    const = tc.alloc_tile_pool(name="const", bufs=1)
    ps = tc.alloc_tile_pool(name="ps", bufs=3, space="PSUM")
    psc = tc.alloc_tile_pool(name="psc", bufs=3, space="PSUM")
    pav = tc.alloc_tile_pool(name="pav", bufs=2, space="PSUM")
    qk_pool = tc.alloc_tile_pool(name="qk", bufs=1)
    v_pool = tc.alloc_tile_pool(name="vp", bufs=1)
    o_pool = tc.alloc_tile_pool(name="op", bufs=1)
    work = tc.alloc_tile_pool(name="wk", bufs=1)
    pool_x = tc.alloc_tile_pool(name="pool_x", bufs=1)

    # ---------------- prologue DMAs (priority order) ----------------
    # pair-0 q-etile weights first (unblocks the very first matmul), x
    # pieces split in half per stream so chunk-0 matmuls start early.
    wq_cur = {}

    def load_wq(which, j, eng=None):
        et = j if which == "q" else NK + j
        for nm in ("r", "i", "s"):
            t = pool_x.tile([128, NK * 128], MM_DT, name=f"w_{nm}_{which}{j}",
                          tag=f"wst_{nm}", bufs=2)
            (eng or nc.sync).dma_start(t[:], io[f"wq_{nm}"][et])
            wq_cur[(which, nm)] = t

    load_wq("q", 0)
    x_sb = {"r": [None, None], "i": [None, None], "s": [None, None]}
    for nm, piece, eng in (("r", 0, nc.scalar), ("i", 0, nc.gpsimd),
                           ("r", 1, nc.scalar), ("i", 1, nc.gpsimd)):
        t = pool_x.tile([128, 4 * S], MM_DT, name=f"x_{nm}_{piece}",
                        tag=f"x{nm}{piece}", bufs=1)
        eng.dma_start(t[:], io[f"x_{nm}_{piece}"][:])
        x_sb[nm][piece] = t
    load_wq("k", 0, eng=nc.scalar)

    # ---------------- constants ----------------
    cos_sb = const.tile([128, S], TAB_DT, name="cos_sb")
    sin_sb = const.tile([128, S], TAB_DT, name="sin_sb")
    cosq_sb = const.tile([128, S], TAB_DT, name="cosq_sb")
    sinq_sb = const.tile([128, S], TAB_DT, name="sinq_sb")
    nc.gpsimd.dma_start(cosq_sb[:], io["cos_q"][:])
    nc.gpsimd.dma_start(sinq_sb[:], io["sin_q"][:])
    nc.gpsimd.dma_start(cos_sb[:], io["cos"][:])
    nc.gpsimd.dma_start(sin_sb[:], io["sin"][:])
    for piece in range(2):
        t = pool_x.tile([128, 4 * S], MM_DT, name=f"x_s_{piece}",
                        tag=f"xs{piece}", bufs=1)
        nc.gpsimd.dma_start(t[:], io[f"x_s_{piece}"][:])
        x_sb["s"][piece] = t
    ones_sb = const.tile([128, 128], MM_DT, name="ones_sb")
    G.memset(ones_sb[:], 1.0)
    b_hpi = const.tile([128, 1], F32, name="b_hpi")
    G.memset(b_hpi[:], math.pi / 2)

    def xch(nm, k):
        return x_sb[nm][k // 4][:, (k % 4) * S:(k % 4 + 1) * S]

    # ---------------- QK projection + RoPE (stacked layout) -------------
    def qk_pair(j):
        """Emit projection + rope for pair j; returns (Mq, Sre, Sim)."""
        res = {}
        for which in ("q", "k"):
            if not (which == "q" and j == 0):
                if not (which == "k" and j == 0):
                    load_wq(which, j)
            w_r = wq_cur[(which, "r")]
            w_i = wq_cur[(which, "i")]
            w_n = wq_cur[(which, "s")]  # holds -wi (4-mult complex)
            p_ar = ps.tile([128, 512], F32, name=f"qk1_{which}{j}", tag="ps")
            p_ai = ps.tile([128, 512], F32, name=f"qk2_{which}{j}", tag="ps")
            for k in range(NK):
                ksl = slice(k * 128, (k + 1) * 128)
                PE.matmul(p_ar[:], w_r[:, ksl], xch("r", k), start=(k == 0),
                          stop=False)
            for k in range(NK):
                ksl = slice(k * 128, (k + 1) * 128)
                PE.matmul(p_ar[:], w_n[:, ksl], xch("i", k), start=False,
                          stop=(k == NK - 1))
            for k in range(NK):
                ksl = slice(k * 128, (k + 1) * 128)
                PE.matmul(p_ai[:], w_i[:, ksl], xch("r", k), start=(k == 0),
                          stop=False)
            for k in range(NK):
                ksl = slice(k * 128, (k + 1) * 128)
                PE.matmul(p_ai[:], w_r[:, ksl], xch("i", k), start=False,
                          stop=(k == NK - 1))
            c_t = cosq_sb if which == "q" else cos_sb
            s_t = sinq_sb if which == "q" else sin_sb
            t1 = work.tile([128, 512], MM_DT, name=f"t1_{which}{j}", tag="t1",
                           bufs=1)
            t2 = work.tile([128, 512], MM_DT, name=f"t2_{which}{j}", tag="t2",
                           bufs=1)
            t3 = work.tile([128, 512], MM_DT, name=f"t3_{which}{j}", tag="t3",
                           bufs=1)
            t4 = work.tile([128, 512], MM_DT, name=f"t4_{which}{j}", tag="t4",
                           bufs=1)
            V.tensor_mul(t1[:], p_ar[:], c_t[:])
            V.tensor_mul(t2[:], p_ai[:], s_t[:])
            V.tensor_mul(t3[:], p_ar[:], s_t[:])
            V.tensor_mul(t4[:], p_ai[:], c_t[:])
            tiles = []
            for half in range(2):
                hs = slice(half * 64, (half + 1) * 64)
                if which == "q":
                    mq = qk_pool.tile([128, 512], MM_DT,
                                      name=f"mq_{j}_{half}", tag="mq", bufs=5)
                    V.tensor_sub(mq[0:64, :], t1[hs, :], t2[hs, :])
                    V.tensor_add(mq[64:128, :], t3[hs, :], t4[hs, :])
                    tiles.append(mq)
                else:
                    sre = qk_pool.tile([128, 512], MM_DT,
                                       name=f"sre_{j}_{half}", tag="sre",
                                       bufs=5)
                    sim = qk_pool.tile([128, 512], MM_DT,
                                       name=f"sim_{j}_{half}", tag="sim",
                                       bufs=5)
                    V.tensor_sub(sre[0:64, :], t1[hs, :], t2[hs, :])
                    V.tensor_add(sre[64:128, :], t3[hs, :], t4[hs, :])
                    V.tensor_scalar_mul(sim[0:64, :], sre[64:128, :], -1.0)
                    V.tensor_scalar_mul(sim[64:128, :], sre[0:64, :], 1.0)
                    tiles.append((sre, sim))
            res[which] = tiles
        return res["q"], [t[0] for t in res["k"]], [t[1] for t in res["k"]]

    # ---------------- V projection into packed AV layouts ----------------
    # v_AB[tt] blocks per head h: even h -> [vr|vi], odd h -> [vi|vr]
    # v_CD[tt] blocks per head h: even h -> [vn|vr], odd h -> [vr|vn]
    v_AB = [v_pool.tile([128, 2048], MM_DT, name=f"vab_{tt}", tag="vab",
                        bufs=NST) for tt in range(NST)]
    v_CD = [v_pool.tile([128, 2048], MM_DT, name=f"vcd_{tt}", tag="vcd",
                        bufs=NST) for tt in range(NST)]

    wv_cur = {}

    def load_wv(n):
        for nm, eng in (("r", nc.gpsimd), ("i", nc.gpsimd), ("s", nc.scalar)):
            t = pool_x.tile([128, NK * 512], MM_DT, name=f"wv_{nm}_{n}",
                          tag=f"wv{nm}", bufs=1)
            eng.dma_start(t[:], io[f"wv_{nm}"][n])
            wv_cur[nm] = t

    def quad4(ap1024, q):
        # [128,1024] -> [128,4,64] selecting 64-col blocks at base q*64,
        # stride 256
        return ap1024.rearrange("p (k q i) -> p k q i", k=4, q=4, i=64)[
            :, :, q, :]

    def half8(ap512, q):
        # psum [128,512] -> [128,4,64] selecting head-blocks at base q*64,
        # stride 128 (q=0: even heads, q=1: odd heads)
        return ap512.rearrange("p (k q i) -> p k q i", k=4, q=2, i=64)[
            :, :, q, :]

    def v_chunk(n, st, pool=None):
        pool = pool or ps
        tag = "ps" if pool is ps else "psc"

        def wvch(nm, k):
            return wv_cur[nm][:, k * 512:(k + 1) * 512]

        ssl = slice(st * 128, (st + 1) * 128)
        p1 = pool.tile([128, 512], F32, name=f"vp1_{st}_{n}", tag=tag)
        p2 = pool.tile([128, 512], F32, name=f"vp2_{st}_{n}", tag=tag)
        p3 = pool.tile([128, 512], F32, name=f"vp3_{st}_{n}", tag=tag)
        for k in range(NK):
            PE.matmul(p1[:], xch("r", k)[:, ssl], wvch("r", k),
                      start=(k == 0), stop=(k == NK - 1))
        for k in range(NK):
            PE.matmul(p2[:], xch("i", k)[:, ssl], wvch("i", k),
                      start=(k == 0), stop=(k == NK - 1))
        for k in range(NK):
            PE.matmul(p3[:], xch("s", k)[:, ssl], wvch("s", k),
                      start=(k == 0), stop=(k == NK - 1))
        ab = v_AB[st][:, n * 1024:(n + 1) * 1024]
        cd = v_CD[st][:, n * 1024:(n + 1) * 1024]
        pc = work.tile([128, 512], F32, name=f"vpc_{st}_{n}", tag="stg",
                       bufs=2)
        V.tensor_scalar_mul(pc[:], p1[:], 1.0)
        # vr = p1 - p2 into AB (even:+0, odd:+64)
        V.tensor_sub(quad4(ab, 0), half8(pc[:], 0), half8(p2[:], 0))
        V.tensor_sub(quad4(ab, 3), half8(pc[:], 1), half8(p2[:], 1))
        # vi = p3 - p1 - p2 into AB (even:+64, odd:+0)
        tf = work.tile([128, 512], F32, name=f"vtf_{st}_{n}", tag="tf",
                       bufs=2)
        V.tensor_sub(tf[:], p3[:], pc[:])
        V.tensor_sub(quad4(ab, 1), half8(tf[:], 0), half8(p2[:], 0))
        V.tensor_sub(quad4(ab, 2), half8(tf[:], 1), half8(p2[:], 1))
        # vr copies into CD (even:+64, odd:+0)
        V.tensor_scalar_mul(quad4(cd, 1), quad4(ab, 0), 1.0)
        V.tensor_scalar_mul(quad4(cd, 2), quad4(ab, 3), 1.0)
        # vn = -vi into CD (even:+0, odd:+64)
        V.tensor_scalar_mul(quad4(cd, 0), quad4(ab, 1), -1.0)
        V.tensor_scalar_mul(quad4(cd, 3), quad4(ab, 2), -1.0)

    # ---------------- output accumulators ----------------
    o_r = [o_pool.tile([128, S], MM_DT, name=f"o_r_{j}", tag="o_r", bufs=NP)
           for j in range(NP)]
    o_i = [o_pool.tile([128, S], MM_DT, name=f"o_i_{j}", tag="o_i", bufs=NP)
           for j in range(NP)]
    o_s = [o_pool.tile([128, S], MM_DT, name=f"o_s_{j}", tag="o_s", bufs=NP)
           for j in range(NP)]

    # ---------------- scores + softmax + AV ----------------
    av_state = {}

    def scores_and_av(j, qk, prev):
        """Emit pair j's scores/softmax; interleave pair prev's denominator
        and AV matmuls into the same PE window."""
        Mq, Sre, Sim = qk
        ebs = [work.tile([128, NST * 512], MM_DT, name=f"eb_{j}_{h}",
                         tag="eb", bufs=2) for h in range(2)]
        phs = [work.tile([128, 2048], F32, name=f"ph_{j}_{h}", tag="ph",
                         bufs=2) for h in range(2)]

        rb_prev = None
        if prev is not None:
            rb_prev = av_state[prev][3]
            ps_o = [pav.tile([128, 512], F32, name=f"av_{prev}_{h}",
                             tag="pav") for h in range(2)]
            Wr_p, Wi_p = av_state[prev][1], av_state[prev][2]

        # scores matmuls (+ AV(prev) interleaved per tt)
        for tt in range(NST):
            tsl = slice(tt * 128, (tt + 1) * 128)
            csl = slice(tt * 512, (tt + 1) * 512)
            for half in range(2):
                ps_re = psc.tile([128, 512], F32, name=f"re_{j}_{half}_{tt}",
                                 tag="psc")
                ps_im = psc.tile([128, 512], F32, name=f"im_{j}_{half}_{tt}",
                                 tag="psc")
                PE.matmul(ps_re[:], Sre[half][:, tsl], Mq[half][:],
                          start=True, stop=True)
                PE.matmul(ps_im[:], Sim[half][:, tsl], Mq[half][:],
                          start=True, stop=True)
                A.activation(ebs[half][:, csl], ps_re[:], AF.Exp,
                             scale=1.0)
                V._custom_dve(ADD_RANGE_WRAP,
                              out=phs[half][:, tt * 512:(tt + 1) * 512],
                              in0=ps_im[:], s0=0.0, s1=math.pi, imm2=TWO_PI)
            if prev is not None:
                h2 = 2 * prev
                for half in range(2):
                    blk = slice((h2 + half) * 128, (h2 + half + 1) * 128)
                    PE.matmul(ps_o[half][:], v_AB[tt][:, blk],
                              Wr_p[half][:, csl], start=(tt == 0), stop=False,
                              skip_group_check=True)
                    PE.matmul(ps_o[half][:], v_CD[tt][:, blk],
                              Wi_p[half][:, csl], start=False,
                              stop=(tt == NST - 1), skip_group_check=True)

        # denominator for THIS pair right after its exps (fills the PE
        # window before trig completes; recip gets a full iteration of lead)
        rb_self = []
        for half in range(2):
            ps_d = psc.tile([128, 512], F32, name=f"dn_{j}_{half}",
                            tag="psc")
            for tt in range(NST):
                PE.matmul(ps_d[:], ones_sb[:],
                          ebs[half][:, tt * 512:(tt + 1) * 512],
                          start=(tt == 0), stop=(tt == NST - 1))
            rb = work.tile([128, 512], F32, name=f"rb_{j}_{half}",
                           tag="rb", bufs=3)
            V.reciprocal_approx_fast(out=rb[:], in_=ps_d[:])
            rb_self.append(rb)
        # trig (single table switch per pair: all exps above, sins below).
        # A value-preserving mark on each ph tile makes every sin's input
        # depend on the pair's last exps, so the scheduler can't interleave
        # exps and sins (would thrash ACT table loads).
        if j < NP - 1:
            tok = work.tile([128, 1], F32, name=f"tok_{j}", tag="tok",
                            bufs=2)
            V.scalar_tensor_tensor(tok[:], ebs[0][:, 2047:2048], 0.0,
                                   ebs[1][:, 2047:2048], ALU.mult, ALU.add)
            for half in range(2):
                for c in range(2):
                    cc = c * 1024
                    V.scalar_tensor_tensor(phs[half][:, cc:cc + 1], tok[:],
                                           0.0, phs[half][:, cc:cc + 1],
                                           ALU.mult, ALU.add)
        s1 = [[None] * 2 for _ in range(2)]
        c1 = [[None] * 2 for _ in range(2)]
        for half in range(2):
            for c in range(2):
                s1t = work.tile([128, 1024], MM_DT, name=f"s1_{j}_{half}_{c}",
                                tag="s1", bufs=2)
                c1t = work.tile([128, 1024], MM_DT, name=f"c1_{j}_{half}_{c}",
                                tag="c1", bufs=2)
                ph = phs[half][:, c * 1024:(c + 1) * 1024]
                A.activation(s1t[:], ph, AF.Sin, scale=SSC)
                A.activation(ph, ph, AF.Abs, scale=1.0)
                A.activation(c1t[:], ph, AF.Sin, bias=b_hpi[:], scale=-1.0)
                s1[half][c] = s1t
                c1[half][c] = c1t
        # W~ = e * (cos, sin); Wr on gpsimd to offload the DVE
        Wr = [work.tile([128, NST * 512], MM_DT, name=f"Wr_{j}_{h}", tag="Wr",
                        bufs=2) for h in range(2)]
        Wi = [work.tile([128, NST * 512], MM_DT, name=f"Wi_{j}_{h}", tag="Wi",
                        bufs=2) for h in range(2)]
        for half in range(2):
            for c in range(2):
                cl = slice(c * 1024, (c + 1) * 1024)
                V.tensor_mul(Wr[half][:, cl], ebs[half][:, cl],
                             c1[half][c][:])
                V.tensor_mul(Wi[half][:, cl], ebs[half][:, cl], s1[half][c][:])

        # AV(prev) eviction
        if prev is not None:
            pj = prev
            V.tensor_mul(o_r[pj][0:64, :], ps_o[0][0:64, :], rb_prev[0][0:64, :])
            V.tensor_mul(o_r[pj][64:128, :], ps_o[1][64:128, :],
                         rb_prev[1][64:128, :])
            V.tensor_mul(o_i[pj][0:64, :], ps_o[1][0:64, :], rb_prev[1][0:64, :])
            V.tensor_mul(o_i[pj][64:128, :], ps_o[0][64:128, :],
                         rb_prev[0][64:128, :])
            osa = work.tile([128, 512], MM_DT, name=f"osa_{pj}", tag="osa",
                            bufs=1)
            V.tensor_scalar_mul(osa[0:64, :], o_i[pj][64:128, :], 1.0)
            V.tensor_scalar_mul(osa[64:128, :], o_i[pj][0:64, :], 1.0)
            V.tensor_add(o_s[pj][:, :], o_r[pj][:, :], osa[:, :])

        av_state[j] = (ebs, Wr, Wi, rb_self)
        if prev is not None:
            del av_state[prev]

    # ---------------- phase 3: output projection (Karatsuba) -------------
    wo_sb = {}

    def load_wo_n0():
        # n=0 halves reuse the wv tag slots (all wv reads done by v(1,3))
        for nm, eng, tag in (("r", nc.sync, "wvr"), ("i", nc.scalar, "wvi"),
                             ("s", nc.gpsimd, "wvs")):
            t = pool_x.tile([128, NK * 512], MM_DT, name=f"wo_{nm}_0",
                            tag=tag, bufs=1)
            eng.dma_start(t[:], io[f"wo_{nm}"][0])
            wo_sb[(nm, 0)] = t

    def load_wo_n1():
        # n=1 halves reuse the x tag slots (x reads done by qk_pair(7))
        slots = {"r": ("xs0", "xs1"), "i": ("xi0", "xi1"),
                 "s": ("xr0", "xr1")}
        for nm, eng in (("r", nc.sync), ("i", nc.gpsimd), ("s", nc.scalar)):
            halves = []
            for piece in range(2):
                t = pool_x.tile([128, 4 * 512], MM_DT,
                                name=f"wo_{nm}_1_{piece}",
                                tag=slots[nm][piece], bufs=1)
                eng.dma_start(t[:], io[f"wo_{nm}"][1][:, piece * 2048:
                                                     (piece + 1) * 2048])
                halves.append(t)
            wo_sb[(nm, 1)] = halves

    def woch(nm, k, n):
        if n == 0:
            return wo_sb[(nm, 0)][:, k * 512:(k + 1) * 512]
        t = wo_sb[(nm, 1)][k // 4]
        kk = k % 4
        return t[:, kk * 512:(kk + 1) * 512]

    def phase3_unit(st, n, pool, upto=NK):
        ssl = slice(st * 128, (st + 1) * 128)
        tag = "ps" if pool is ps else "psc"
        p1 = pool.tile([128, 512], F32, name=f"pj1_{st}_{n}", tag=tag)
        p2 = pool.tile([128, 512], F32, name=f"pj2_{st}_{n}", tag=tag)
        p3 = pool.tile([128, 512], F32, name=f"pj3_{st}_{n}", tag=tag)
        state = {"k": 0}

        def advance(upto2):
            for k in range(state["k"], upto2):
                PE.matmul(p1[:], o_r[k][:, ssl], woch("r", k, n),
                          start=(k == 0), stop=(k == NK - 1))
                PE.matmul(p2[:], o_i[k][:, ssl], woch("i", k, n),
                          start=(k == 0), stop=(k == NK - 1))
                PE.matmul(p3[:], o_s[k][:, ssl], woch("s", k, n),
                          start=(k == 0), stop=(k == NK - 1))
            state["k"] = upto2

        def finish():
            advance(NK)
            to_r = work.tile([128, 512], MM_DT, name=f"otr_{st}_{n}",
                             tag="out_r", bufs=1)
            to_i = work.tile([128, 512], MM_DT, name=f"oti_{st}_{n}",
                             tag="out_i", bufs=1)
            tf3 = work.tile([128, 512], F32, name=f"otf_{st}_{n}",
                            tag="out_f", bufs=1)
            pc3 = work.tile([128, 512], F32, name=f"opc_{st}_{n}",
                            tag="stg", bufs=2)
            V.tensor_scalar_mul(pc3[:], p1[:], 1.0)
            V.tensor_sub(to_r[:], pc3[:], p2[:])
            V.tensor_sub(tf3[:], p3[:], pc3[:])
            V.tensor_sub(to_i[:], tf3[:], p2[:])
            nsl = slice(n * 512, (n + 1) * 512)
            nc.sync.dma_start(out[0, ssl, nsl], to_r[:])
            nc.sync.dma_start(out[1, ssl, nsl], to_i[:])

        advance(upto)
        return advance, finish

    # ---------------- emission schedule ----------------
    load_wv(0)
    qk0 = qk_pair(0)
    qk1 = qk_pair(1)
    scores_and_av(0, qk0, None)
    for st in range(NST):
        v_chunk(0, st)
    qk_tiles = {1: qk1}
    pre_units = []
    for j in range(1, NP):
        if j + 1 < NP:
            qk_tiles[j + 1] = qk_pair(j + 1)
            if j + 1 == NP - 1:
                load_wo_n1()
        if j == 5:
            load_wo_n0()
        if j == NP - 1:
            # fill the last pair's softmax latency with phase-3 partials
            # (ps pool only - psc is still needed by scores/denom)
            pre_units.append(phase3_unit(0, 0, ps, upto=NK - 2))
        scores_and_av(j, qk_tiles.pop(j), j - 1)
        if j == 1:
            load_wv(1)
        if 1 <= j <= NST:
            v_chunk(1, j - 1)

    # last pair's denominator+AV, interleaved with the first phase-3 units
    for adv, _f in pre_units:
        adv(NK - 1)
    # emit AV for pair 7 (denominator already computed in its scores pass)
    prev = NP - 1
    rb_prev = av_state[prev][3]
    pre_units.append(phase3_unit(1, 0, psc, upto=NK - 1))
    ps_o = [pav.tile([128, 512], F32, name=f"av_{prev}_{h}", tag="pav")
            for h in range(2)]
    Wr_p, Wi_p = av_state[prev][1], av_state[prev][2]
    h2 = 2 * prev
    for tt in range(NST):
        csl = slice(tt * 512, (tt + 1) * 512)
        for half in range(2):
            blk = slice((h2 + half) * 128, (h2 + half + 1) * 128)
            PE.matmul(ps_o[half][:], v_AB[tt][:, blk], Wr_p[half][:, csl],
                      start=(tt == 0), stop=False, skip_group_check=True)
            PE.matmul(ps_o[half][:], v_CD[tt][:, blk], Wi_p[half][:, csl],
                      start=False, stop=(tt == NST - 1),
                      skip_group_check=True)
    pj = prev
    V.tensor_mul(o_r[pj][0:64, :], ps_o[0][0:64, :], rb_prev[0][0:64, :])
    V.tensor_mul(o_r[pj][64:128, :], ps_o[1][64:128, :], rb_prev[1][64:128, :])
    V.tensor_mul(o_i[pj][0:64, :], ps_o[1][0:64, :], rb_prev[1][0:64, :])
    V.tensor_mul(o_i[pj][64:128, :], ps_o[0][64:128, :], rb_prev[0][64:128, :])
    osa = work.tile([128, 512], MM_DT, name=f"osa_{pj}", tag="osa", bufs=1)
    V.tensor_scalar_mul(osa[0:64, :], o_i[pj][64:128, :], 1.0)
    V.tensor_scalar_mul(osa[64:128, :], o_i[pj][0:64, :], 1.0)
    V.tensor_add(o_s[pj][:, :], o_r[pj][:, :], osa[:, :])
    del av_state[prev]

    for _adv, fin in pre_units:
        fin()
    pools3 = [ps, psc]
    i3 = 0
    for n in range(2):
        for st in range(NST):
            if st <= 1 and n == 0:
                continue
            _a, fin = phase3_unit(st, n, pools3[i3 % 2])
            fin()
            i3 += 1

    for p in (pool_x, work, o_pool, v_pool, qk_pool, pav, psc, ps, const):
        p.release()


def _install_act_root():
    """Restrict walrus to the {exp_and_others, trig_and_small} ACT table
    sets so exp and sin are each one load away and nothing else thrashes.
    On any failure, degrade to the default tables (correct, slower)."""
    if os.environ.get("K_NO_ACTFIX"):
        return
    if os.environ.get("BASS_ACT_ROOT_JSON_PATH"):
        return
    try:
        _install_act_root_impl()
    except Exception:
        os.environ["K_NO_ACTFIX"] = "1"


_KEEP_SETS = ("exp_and_others", "trig_and_small")


def _install_act_root_impl():
    import json
    import tempfile
    from neuronxcc.driver.Job import Job
    from neuronxcc.driver.jobs.support.FindActInfo import findActInfoFile

    p = findActInfoFile(Job.getPackageDir(), "gen3")
    src_dir = os.path.dirname(p)
    with open(p) as f:
        d = json.load(f)
    d["act_func_sets"] = [e for e in d["act_func_sets"]
                          if e["name"] in _KEEP_SETS]
    out_dir = tempfile.mkdtemp(prefix="act_expsin_")
    for fn in os.listdir(src_dir):
        sp = os.path.join(src_dir, fn)
        if os.path.isfile(sp) and fn != "act_info.json":
            os.symlink(sp, os.path.join(out_dir, fn))
    with open(os.path.join(out_dir, "act_info.json"), "w") as f:
        json.dump(d, f)
    os.environ["BASS_ACT_ROOT_JSON_PATH"] = os.path.join(out_dir,
                                                         "act_info.json")
    import concourse.hw_specs as hw_specs
    import concourse.bacc as bacc_mod

    orig = hw_specs.get_activation_tables.__wrapped__

    @__import__("functools").cache
    def only_kept(arch):
        full = orig(arch)
        return {k: full[k] for k in _KEEP_SETS}

    hw_specs.get_activation_tables = only_kept
    bacc_mod.get_activation_tables = only_kept


def build_nc():
    _install_act_root()
    nc = bacc.Bacc("TRN2", target_bir_lowering=False, debug=False,
                   enable_asserts=False, num_devices=8)
    io = {}

    def inp(name, shape, dt=MM_DT):
        io[name] = nc.dram_tensor(name, shape, dt, kind="ExternalInput").ap()

    for nm in ("r", "i", "s"):
        inp(f"x_{nm}_0", [128, 4 * S])
        inp(f"x_{nm}_1", [128, 4 * S])
        inp(f"wq_{nm}", [2 * NK, 128, NK * 128])
        inp(f"wv_{nm}", [2, 128, NK * 512])
        inp(f"wo_{nm}", [2, 128, NK * 512])
    inp("cos", [128, S], TAB_DT)
    inp("sin", [128, S], TAB_DT)
    inp("cos_q", [128, S], TAB_DT)
    inp("sin_q", [128, S], TAB_DT)
    io["out"] = nc.dram_tensor("out", [2, S, DM], MM_DT,
                               kind="ExternalOutput").ap()

    with tile.TileContext(nc) as tc:
        build_body(nc, tc, io)
    nc.compile()
    return nc


def host_inputs(xr, xi, wqkv_r, wqkv_i, wo_r, wo_i):
    """Pack full f32 inputs into 8 per-core in_maps."""
    np_mm = mybir.dt.np(MM_DT)
    np_tab = mybir.dt.np(TAB_DT)

    def pack_qk(w):  # (D, 3D) -> [16e][128p][8k*128]
        return np.ascontiguousarray(
            w[:, :2 * DM].reshape(NK, 128, 2 * NK, 128).transpose(2, 1, 0, 3)
            .reshape(2 * NK, 128, NK * 128))

    def pack_v(w):  # -> [2n][128p][8k*512]
        return np.ascontiguousarray(
            w[:, 2 * DM:].reshape(NK, 128, 2, 512).transpose(2, 1, 0, 3)
            .reshape(2, 128, NK * 512))

    def pack_p(w):  # (NK,128,F) row-major -> [128p][NK*F]
        return np.ascontiguousarray(
            w.transpose(1, 0, 2).reshape(128, -1))

    def pack_wo(w):  # (D=e, D=out) -> [2n][128p][NK*512]
        return np.ascontiguousarray(
            w.reshape(NK, 128, 2, 512).transpose(2, 1, 0, 3)
            .reshape(2, 128, NK * 512))

    wqkvT_r = np.ascontiguousarray(wqkv_r.T).astype(np_mm)  # (D, 3D)
    wqkvT_i = np.ascontiguousarray(wqkv_i.T).astype(np_mm)
    wqkvT_s = (wqkvT_r.astype(np.float32)
               + wqkvT_i.astype(np.float32)).astype(np_mm)
    woT_r = np.ascontiguousarray(wo_r.T.astype(np_mm))  # (D_in=e, D_out)
    woT_i = np.ascontiguousarray(wo_i.T.astype(np_mm))
    woT_s = (woT_r.astype(np.float32)
             + woT_i.astype(np.float32)).astype(np_mm)
    # o_i e-rows come out of the AV pass pair-swapped: permute wo_i rows to
    # match ([h1|h0] within each pair)
    perm = np.arange(DM).reshape(NP, 2, DH)[:, ::-1, :].reshape(DM)
    woT_i = np.ascontiguousarray(woT_i[perm])

    inv_freq = 1.0 / (10000.0 ** (np.arange(DH, dtype=np.float64) / DH))
    ang = np.arange(S, dtype=np.float64)[:, None] * inv_freq[None, :]  # (S,Dh)
    cosT = np.cos(ang).T  # (Dh, S)
    sinT = np.sin(ang).T

    def dup(t):
        return np.ascontiguousarray(np.concatenate([t, t], axis=0))

    shared = {
        "wq_r": pack_qk(wqkvT_r), "wq_i": pack_qk(wqkvT_i),
        "wq_s": pack_qk(-wqkvT_i.astype(np.float32)).astype(np_mm),
        "wv_r": pack_v(wqkvT_r), "wv_i": pack_v(wqkvT_i),
        "wv_s": pack_v(wqkvT_s),
        "wo_r": pack_wo(woT_r), "wo_i": pack_wo(woT_i),
        "wo_s": pack_wo(woT_s),
        "cos": dup(cosT).astype(np_tab), "sin": dup(sinT).astype(np_tab),
        "cos_q": (dup(cosT) * SCALE).astype(np_tab),
        "sin_q": (dup(sinT) * SCALE).astype(np_tab),
    }
    in_maps = []
    for b in range(B):
        xT_r = xr[b].T.astype(np_mm).reshape(NK, 128, S)
        xT_i = xi[b].T.astype(np_mm).reshape(NK, 128, S)
        xT_s = (xT_r.astype(np.float32)
                + xT_i.astype(np.float32)).astype(np_mm)
        m = {}
        for nm, t in (("r", xT_r), ("i", xT_i), ("s", xT_s)):
            packed = pack_p(t)  # [128, NK*S]
            m[f"x_{nm}_0"] = np.ascontiguousarray(packed[:, :4 * S])
            m[f"x_{nm}_1"] = np.ascontiguousarray(packed[:, 4 * S:])
        m.update(shared)
        in_maps.append(m)
    return in_maps


_NC_CACHE = None


def get_nc():
    global _NC_CACHE
    if _NC_CACHE is None:
        _NC_CACHE = build_nc()
    return _NC_CACHE


def kernel(xr, xi, wqkv_r, wqkv_i, wo_r, wo_i):
    from concourse.bass_utils import run_bass_kernel_spmd

    _install_act_root()
    in_maps = host_inputs(np.asarray(xr, np.float32),
                          np.asarray(xi, np.float32),
                          np.asarray(wqkv_r, np.float32),
                          np.asarray(wqkv_i, np.float32),
                          np.asarray(wo_r, np.float32),
                          np.asarray(wo_i, np.float32))
    nc = get_nc()
    res = run_bass_kernel_spmd(nc, in_maps, core_ids=list(range(B)),
                               trace=bool(int(os.environ.get("K_TRACE", "0"))))
    out_r = np.stack([res.results[b]["out"][0].astype(np.float32)
                      for b in range(B)])
    out_i = np.stack([res.results[b]["out"][1].astype(np.float32)
                      for b in range(B)])
    kernel.last_results = res
    return out_r, out_i
